# revision 19
# baseline (speedup 1.0000x reference)
"""Multi-head attention (softmax over query axis) on 8 Trainium2 cores.

Problem: nn_MultiHeadAttention_3899830305178
  B=2, S=2048, D_MODEL=1024, HEADS=16, D_K=64, fp32 IO.
  reference:
    q = (query @ Wq + bq), k = ..., v = ...        [b, s, h, dk]
    scores = einsum('bihd,bjhd->bijh', q, k) / 8
    attn = softmax(scores, axis=1)                 # over QUERY axis i (quirk)
    x = einsum('bijh,bjhd->bihd', attn, v)         [b, s, h*dk]
    out = x @ Wo + bo

Sharding: data-parallel over batch (2) x tensor-parallel over heads (4 groups
of 4 heads) = 8 cores. Each core computes a partial output
O_part = x_local @ Wo[rows of its heads]; the host sums the 4 partials per
batch (row-parallel unshard) -- bo is added on-device by the g==0 core.

Per-core kernel math (host passes query/key/value pre-transposed so the
projections contract over the model dim on partitions):
  qT[d', i] = Wq_s.T @ queryT      (d' = 4 local heads x 64 = 256)
  kT[d', j] = Wk_s.T @ keyT
  vT[d', j] = Wv_s.T @ valueT (+bv), then bf16 DMA-transpose -> v[j, d']
  per head h:  sT[j, i] = kT_h.T @ qT_h / 8  (softmax over i == free axis)
               eT = exp(sT) (bf16), rowsum via a DVE tensor_scalar accum
               v_h_scaled[j, :] = v_h[j, :] / rowsum[j]   <- softmax divisor
               xT_h[d, i] = v_h_scaled.T @ eT             (contracts over j)
  O_part[i, n] = xT.T @ Wo_s; bo is added on the host during unshard.

Engine balance (per CoreSim, 230us single-shot span): PE ~169us
(projections 41 + scores 55 + attn@V 55 + out-proj 14), ACT ~138us (the
128 [128,1024] exps are irreducible -- Exp exists only on ACT), DVE
~93us, 16KB/partition PSUM exactly full (2x scores buffers + 2x
proj/attn@V buffers). Startup: wq/wk load on the (initially idle) ACT
HWDGE queue in parallel with the q/k input chunks on SP; the q/k
projection stream is ordered q0,q1,k0,q2,q3,k1..k3 to match the ACT
engine's exp demand order; wv/wo load late; qT is tiled per input chunk
so each scores matmul gates on exactly one projected chunk.
Design choices vs the naive version:
  - sibling heads (partitions 0-63 / 64-127 of the kT/qT slices) emit
    their K=64 scores matmuls interleaved with PE tile positions
    (0,0)/(64,0), letting the PE overlap row tiles (K=64 alone half-fills
    the 128x128 array);
  - eT is bf16 (same PE rate as f32r, half the SBUF/attp footprint);
  - softmax rowsums come from a DVE tensor_scalar (2-byte fast mode,
    ~0.4us/tile) instead of the ACT accum_out (+187ns/tile on the other
    near-critical engine), computed as out=(a*1)+0 in place with
    accum_out=rowsum;
  - attn@V is split by i-half into [64,1024] psums so the first half's
    output projection overlaps the second half's attention (xT is split
    per i-half so the dependency is tile-precise); pair 1's first scores
    are emitted inside pair 0's phase B so the ACT engine never idles
    there, and the first-half output projection (with O DMAs alternating
    between the SP and ACT hardware queues) drains during pair 1's
    phase B;
  - the bias matmuls (K=1 ones-row) were removed from the PE: bo rides
    the host-side unshard sum.

Projection inputs/weights are bf16; scores and the output projection run
in float32r (TF32, fp32 accumulate) with fp32 softmax statistics; attn@V
is bf16 x bf16 -> fp32. Measured end-to-end relative error vs the fp64
reference is ~4.4e-3 on hardware.
"""

import numpy as np

import concourse.bass as bass
import concourse.mybir as mybir
import concourse.tile as tile
from concourse.bass_utils import run_bass_kernel_spmd

# problem shape (hardcoded per contract)
B, S, DM, H, DK = 2, 2048, 1024, 16, 64
N_CORES = 8
GROUPS = 4              # head groups (tensor-parallel)
HL = H // GROUPS        # 4 local heads per core
DL = HL * DK            # 256 local concat width
P = 128
SJ = S // P             # 16 strips of 128 along j (keys) and i (out rows)
MT = DM // P            # 8 contraction tiles for projections
DPT = DL // P           # 2 partition tiles of the local concat dim
SCALE = 1.0 / 8.0       # 1/sqrt(DK)

f32 = mybir.dt.float32
f32r = mybir.dt.float32r
bf16 = mybir.dt.bfloat16
AF = mybir.ActivationFunctionType

# Projection stage (inputs + projection weights) in bf16: halves the input
# DMA (the critical-path prefix) at ~2e-3 relative error. Attention and
# output projection stay TF32.
PROJ_BF16 = True
PROJ_DT = bf16 if PROJ_BF16 else f32r

import os as _os
# Reuse PE stationary weights across same-lhsT matmul runs by suppressing
# the per-matmul LDWEIGHTS (InstMatmult.ldweights=False on the trailing
# matmuls of each run).
LDW_REUSE = _os.environ.get("LDW_REUSE", "1") == "1"
# Which j parity routes its softmax rowsums to the DVE (the other parity
# uses the exp's ACT-side accumulator). 2 = all rowsums on ACT.
ROWSUM_DVE_PARITY = int(_os.environ.get("ROWSUM_DVE_PARITY", "1"))

_PROGRAM = None


def _dedupe_ldweights(nc):
    """Drop InstLdweights that reload the exact weights already resident.

    Tile's legalizer splits every matmul into (InstLdweights, InstMatmult
    ldweights=False); each reload costs ~120-180ns serialized into the PE
    stream. When consecutive LDWEIGHTS on the PE stream have identical
    weight APs (the kernel emits same-lhsT matmul runs for scores, attn@V
    and the output projection), the duplicates are pure overhead: the
    array still holds the weights (nothing else writes it), and the WAR
    protection on the SBUF region anchors on the matmuls (Tile tracked
    them as the lhsT readers), so dropping the reload is safe. Waits and
    sem updates of a dropped LDWEIGHTS move onto a NOP in its place."""
    n = 0
    for f in nc.m.functions:
        for blk in f.blocks:
            last_key = None
            new_insts = []
            for inst in blk.instructions:
                if getattr(inst, "engine", None) == mybir.EngineType.PE:
                    tn = type(inst).__name__
                    if tn == "InstLdweights":
                        key = (str(inst.ins[0]), str(inst.tile_position),
                               str(inst.perf_mode), str(inst.is_transpose))
                        if key == last_key:
                            si = inst.sync_info
                            if si is not None and (si.on_wait or si.on_update):
                                new_insts.append(mybir.InstNoOp(
                                    name=f"{inst.name}-ldwskip",
                                    engine=inst.engine,
                                    sync_info=si,
                                    bass_nofuse=True))
                            n += 1
                            continue
                        last_key = key
                    elif tn == "InstMatmult":
                        if inst.is_transpose:
                            last_key = None
                    elif tn in ("InstNoOp", "InstEventSemaphore"):
                        pass
                    else:
                        last_key = None
                new_insts.append(inst)
            blk.instructions[:] = new_insts
    return n


def _split_excess_waits(nc, max_waits=1):
    """walrus in this container rejects >1 semaphore wait per instruction
    (e.g. the Tile kernel-tail Drain); move extras onto same-engine NOPs."""
    n_split = 0
    for f in nc.m.functions:
        for blk in f.blocks:
            new_insts = []
            for inst in blk.instructions:
                si = getattr(inst, "sync_info", None)
                if si is not None and si.on_wait and len(si.on_wait) > max_waits:
                    waits = list(si.on_wait)
                    extra, keep = waits[:-max_waits], waits[-max_waits:]
                    for i in range(0, len(extra), max_waits):
                        chunk = extra[i:i + max_waits]
                        nop = mybir.InstNoOp(
                            name=f"{inst.name}-ws{n_split}-{i}",
                            engine=inst.engine,
                            sync_info=mybir.SyncInfo(on_wait=chunk, on_update=[]),
                            bass_nofuse=True,
                        )
                        new_insts.append(nop)
                    si.on_wait = keep
                    n_split += 1
                new_insts.append(inst)
            blk.instructions[:] = new_insts
    return n_split


QK_ORDER = [("q", 0), ("k", 0), ("q", 1), ("q", 2), ("q", 3),
            ("k", 1), ("k", 2), ("k", 3)]


def emit_projections(nc, tc, it, const, sb, inp, vtp, ppx,
                     qT_in, kT_in, vT_in,
                     wq_sb, wk_sb, load_wv, bq_sb, bk_sb, bv_sb, w_slice):
    """Create rep-it's activation tiles and return (state, chunk closures).

    Each closure emits one input chunk's DMA + projection matmuls + bias
    adds (and for v, the v4 transposes). The caller interleaves the
    closures of rep it+1 into rep it's attention j-loop so the static
    scheduler places the projection matmuls inside the attention span
    (the PE has ~2x headroom there); without this the scheduler abuts the
    reps and the ACT stream stalls ~50-60us per rep waiting for
    projections."""
    R = f"_r{it}"
    # bufs=2 on qT/kT/v4: the next rep's projections write the other
    # buffer while this rep's attention still reads this one.
    qT_sb = [[sb.tile([P, 512], bf16, name=f"qT{dp}_{i4}{R}",
                      tag=f"qT{dp}_{i4}", bufs=2) for i4 in range(4)]
             for dp in range(DPT)]
    kT_sb = [[sb.tile([P, 512], bf16, name=f"kT{dp}_{jg}{R}",
                      tag=f"kT{dp}_{jg}", bufs=2) for jg in range(4)]
             for dp in range(DPT)]
    # v packed per j-group of 4: v4_sb[jg][p, jj*DL + d'] holds
    # v[jg*512 + jj*128 + p, d']
    v4_sb = [sb.tile([P, 4 * DL], bf16, name=f"v{jg}{R}", tag=f"v{jg}",
                     bufs=2)
             for jg in range(4)]
    xT_sb = [[sb.tile([P, 1024], bf16, name=f"xT{hp}_{ih}{R}",
                      tag=f"xT{hp}_{ih}") for ih in range(2)]
             for hp in range(DPT)]
    vT_sb = [vtp.tile([P, S], bf16, name=f"vT{dp}{R}", tag=f"vT{dp}")
             for dp in range(DPT)]

    def load_in_chunk(win, nm, i4):
        # one DMA: all 8 m-blocks of columns [i0, i0+512)
        t = inp.tile([P, MT * 512], PROJ_DT, name=f"{nm}in{i4}{R}",
                     tag="pin")
        src = win.ap().rearrange("(t p) c -> p t c", p=P)
        nc.sync.dma_start(
            t[:].rearrange("p (t c) -> p t c", t=MT),
            src[:, :, i4 * 512:(i4 + 1) * 512])
        return t

    def qk_chunk(nm, i4):
        def emit():
            win, w_sb, b_sb = ((qT_in, wq_sb, bq_sb) if nm == "q"
                               else (kT_in, wk_sb, bk_sb))
            ch = load_in_chunk(win, nm, i4)
            for dp in range(DPT):
                ps = ppx.tile([P, 512], f32, name=f"ps{nm}{i4}_{dp}{R}",
                              tag="px", bufs=2)
                for m in range(MT):
                    nc.tensor.matmul(
                        ps[:], w_slice(w_sb, m, dp),
                        ch[:, m * 512:(m + 1) * 512],
                        start=(m == 0), stop=(m == MT - 1))
                dst = (qT_sb if nm == "q" else kT_sb)[dp][i4][:]
                nc.vector.tensor_scalar_add(dst, ps[:], b_sb[:, dp:dp + 1])
        return emit

    def v_chunk(i4):
        def emit():
            wv_sb = load_wv()
            i0 = i4 * 512
            ch = load_in_chunk(vT_in, "v", i4)
            for dp in range(DPT):
                ps = ppx.tile([P, 512], f32, name=f"psvt{i4}_{dp}{R}",
                              tag="px", bufs=2)
                for m in range(MT):
                    nc.tensor.matmul(
                        ps[:], w_slice(wv_sb, m, dp),
                        ch[:, m * 512:(m + 1) * 512],
                        start=(m == 0), stop=(m == MT - 1))
                nc.vector.tensor_scalar_add(
                    vT_sb[dp][:, i0:i0 + 512], ps[:], bv_sb[:, dp:dp + 1])
            for dp in range(DPT):
                out_view = v4_sb[i4][:].rearrange(
                    "p (j c) -> p j c", j=4)[:, :, dp * P:(dp + 1) * P]
                nc.sync.dma_start(
                    out_view, vT_sb[dp][:, i0:i0 + 512], transpose=True)
        return emit

    closures = [qk_chunk(nm, i4) for nm, i4 in QK_ORDER]
    closures += [v_chunk(i4) for i4 in range(4)]
    st = dict(qT_sb=qT_sb, kT_sb=kT_sb, v4_sb=v4_sb, xT_sb=xT_sb)
    return st, closures


def emit_attention(nc, tc, it, st, feeder, const, stat, outp, attp, pps,
                   ppx, wo_d, O_d, wo_cell):
    """Attention + output projection for rep it; `feeder` holds the next
    rep's projection-chunk closures, interleaved into the j-loop."""
    R = f"_r{it}"
    qT_sb, kT_sb = st["qT_sb"], st["kT_sb"]
    v4_sb, xT_sb = st["v4_sb"], st["xT_sb"]

    # ---------------- attention ----------------
    # Heads run in sibling pairs (2hp, 2hp+1) whose kT/qT slices live at
    # partitions 0-63 / 64-127 (PE row tiles 0/64). Per (head, j) the four
    # scores matmuls (2 i-halves x 2 i-chunks) share one kT stationary
    # slice and the four attn@V matmuls share one vsc slice: with
    # LDW_REUSE the trailing matmuls set InstMatmult.ldweights=False so
    # walrus skips the per-matmul LDWEIGHTS reload (HW trace showed the
    # 768 reloads/rep serialize ~130ns each into the PE stream).
    # Softmax rowsums ride the exp's ACT-side accumulator on even j and a
    # DVE tensor_scalar on odd j, balancing the two near-critical engines.
    # attn@V accumulates both i-halves into two [128, 1024] psums held for
    # the whole pair (sibling heads at psum partitions 0-63/64-127).

    MULT = mybir.AluOpType.mult
    ADD = mybir.AluOpType.add

    def head_scores(hp, hh, j):
        """scores + exp + rowsum for one head, full i range (2 psum tiles)."""
        jg, jr = divmod(j, 4)
        h = hp * 2 + hh
        base = hh * 64
        lhs = kT_sb[hp][jg][base:base + 64, jr * P:(jr + 1) * P]
        pss = []
        first = True
        for ih in range(2):
            ps = pps.tile([P, 1024], f32, name=f"ps{h}_{j}_{ih}{R}",
                          tag="ps")
            for i5 in range(2):
                mm = nc.tensor.matmul(
                    ps[:, i5 * 512:(i5 + 1) * 512], lhs,
                    qT_sb[hp][ih * 2 + i5][base:base + 64, :],
                    start=True, stop=True)
                if LDW_REUSE and not first:
                    mm.ins.ldweights = False
                first = False
            pss.append(ps)
        outs = []
        for ih, ps in enumerate(pss):
            a = attp.tile([P, 1024], bf16, name=f"att{h}_{j}_{ih}{R}",
                          tag=f"att{ih}", bufs=(12 if ih == 0 else 33))
            rsh = stat.tile([P, 1], f32, name=f"rsh{h}_{j}_{ih}{R}",
                            tag="rsh", bufs=16)
            if j % 2 == ROWSUM_DVE_PARITY:
                nc.scalar.activation(a[:], ps[:], AF.Exp, scale=SCALE)
                nc.vector.tensor_scalar(a[:], a[:], 1.0, 0.0, MULT, ADD,
                                        accum_out=rsh[:])
            else:
                nc.scalar.activation(a[:], ps[:], AF.Exp, scale=SCALE,
                                     accum_out=rsh[:])
            outs.append((a, rsh))
        return outs

    def head_finish_a(hp, hh, j, xph, outs):
        """softmax divisor onto v, then attn@V for i-half 0; saves the
        i-half-1 eT and the vsc tile for phase B."""
        h = hp * 2 + hh
        jg, jr = divmod(j, 4)
        rs = stat.tile([P, 1], f32, name=f"rs{h}_{j}{R}", tag="rs")
        nc.vector.tensor_add(rs[:], outs[0][1][:], outs[1][1][:])
        rc = stat.tile([P, 1], f32, name=f"rc{h}_{j}{R}", tag="rc")
        nc.vector.reciprocal(rc[:], rs[:])
        vsc = attp.tile([P, 64], bf16, name=f"vsc{h}_{j}{R}", tag="vsc",
                        bufs=34)
        nc.vector.tensor_scalar_mul(
            vsc[:],
            v4_sb[jg][:, jr * DL + h * 64:jr * DL + (h + 1) * 64],
            rc[:])
        vsc_t[h][j] = vsc
        a1_t[h][j] = outs[1][0]
        for i5 in range(2):
            nc.tensor.matmul(
                xph[hh * 64:(hh + 1) * 64, i5 * 512:(i5 + 1) * 512],
                vsc[:], outs[0][0][:, i5 * 512:(i5 + 1) * 512],
                start=(j == 0), stop=(j == SJ - 1),
                skip_group_check=True)

    def head_finish_b(hp, hh, j, xph):
        """attn@V for i-half 1 from the saved eT/vsc (pure PE work)."""
        h = hp * 2 + hh
        a = a1_t[h][j]
        for i5 in range(2):
            nc.tensor.matmul(
                xph[hh * 64:(hh + 1) * 64, i5 * 512:(i5 + 1) * 512],
                vsc_t[h][j][:], a[:, i5 * 512:(i5 + 1) * 512],
                start=(j == 0), stop=(j == SJ - 1),
                skip_group_check=True)

    vsc_t = [[None] * SJ for _ in range(HL)]
    a1_t = [[None] * SJ for _ in range(HL)]

    # ---------------- output projection constants ----------------
    # bo is added on the host during unshard (a K=1 ones-row matmul for it
    # here would cost 16384 PE rows ~ 7us).
    if not wo_cell:
        wo_sb = const.tile([P, DPT * DM], bf16, name="wo", tag="wo")
        nc.sync.dma_start(
            wo_sb[:].rearrange("p (t c) -> p t c", t=DPT),
            wo_d.ap().rearrange("(t p) c -> p t c", p=P))
        wo_cell.append(wo_sb)
    wo_sb = wo_cell[0]

    def emit_outproj(jts):
        for jt in jts:
            ot = outp.tile([P, DM], f32, name=f"ot{jt}{R}", tag="ot")
            ps = ppx.tile([P, DM], f32, name=f"pso{jt}{R}", tag="xps",
                          bufs=1)
            jh, jo = divmod(jt, 8)
            for cpt in range(DPT):
                first = True
                for n5 in range(2):
                    no = n5 * 512
                    mm = nc.tensor.matmul(
                        ps[:, no:no + 512],
                        xT_sb[cpt][jh][:, jo * P:(jo + 1) * P],
                        wo_sb[:, cpt * DM + no:cpt * DM + no + 512],
                        start=(cpt == 0), stop=(cpt == DPT - 1))
                    if LDW_REUSE and not first:
                        mm.ins.ldweights = False
                    first = False
            nc.vector.tensor_copy(ot[:], ps[:])
            oq = nc.sync if jt % 2 == 0 else nc.gpsimd
            oq.dma_start(O_d.ap()[jt * P:(jt + 1) * P, :], ot[:])

    def alloc_xps(hp, ih):
        return ppx.tile([P, 1024], f32, name=f"xp{ih}_{hp}{R}", tag="xps",
                        bufs=1)

    def copy_xps(hp, ih, xph):
        nc.vector.tensor_copy(xT_sb[hp][ih][:], xph[:])

    prio = tc.high_priority()
    prio.__enter__()

    # Feed points: (hp, phase, j) -> chunk index of the NEXT rep's
    # projections. Spread so the PE digests one ~3.5us chunk per ~2 j's.
    feed_at = {}
    for ci, (hp, ph, j) in enumerate(
            [(0, 0, 2), (0, 0, 4), (0, 0, 6), (0, 0, 8), (0, 0, 10),
             (0, 0, 12), (0, 0, 14), (0, 1, 4), (0, 1, 10),
             (1, 0, 1), (1, 0, 4), (1, 0, 7)]):
        feed_at[(hp, ph, j)] = ci

    def feed(hp, ph, j):
        ci = feed_at.get((hp, ph, j))
        if ci is not None and ci < len(feeder):
            feeder[ci]()

    # Per pair: phase A runs scores + ALL exps (both i-halves; the rowsum
    # needs the full i range) plus attn@V for i-half 0; phase B is the
    # pure-PE attn@V for i-half 1 from saved eT/vsc tiles. The single-slot
    # "xps" psum rotation (A -> B -> next pair) leaves 2 banks for the
    # next rep's projections to overlap this rep's attention.
    for hp in range(2):
        xph = alloc_xps(hp, 0)
        for j in range(SJ):
            oA = head_scores(hp, 0, j)
            oB = head_scores(hp, 1, j)
            head_finish_a(hp, 0, j, xph, oA)
            head_finish_a(hp, 1, j, xph, oB)
            feed(hp, 0, j)
        copy_xps(hp, 0, xph)
        xph = alloc_xps(hp, 1)
        for j in range(SJ):
            head_finish_b(hp, 0, j, xph)
            head_finish_b(hp, 1, j, xph)
            feed(hp, 1, j)
        copy_xps(hp, 1, xph)
    prio.__exit__(None, None, None)

    # output projection trails the whole rep: pure PE+DVE+DMA work that
    # overlaps the next rep's projections (which own the high-prio lane).
    emit_outproj(range(SJ))


def build_program(split_waits=True, reps=1):
    nc = bass.Bass("TRN2", target_bir_lowering=False, debug=False)

    qT_in = nc.dram_tensor("qT_in", [DM, S], PROJ_DT, kind="ExternalInput")
    kT_in = nc.dram_tensor("kT_in", [DM, S], PROJ_DT, kind="ExternalInput")
    vT_in = nc.dram_tensor("vT_in", [DM, S], PROJ_DT, kind="ExternalInput")
    wq_d = nc.dram_tensor("wq", [DM, DL], PROJ_DT, kind="ExternalInput")
    wk_d = nc.dram_tensor("wk", [DM, DL], PROJ_DT, kind="ExternalInput")
    wv_d = nc.dram_tensor("wv", [DM, DL], PROJ_DT, kind="ExternalInput")
    wo_d = nc.dram_tensor("wo", [DL, DM], bf16, kind="ExternalInput")
    bq_d = nc.dram_tensor("bq", [DL, 1], f32, kind="ExternalInput")
    bk_d = nc.dram_tensor("bk", [DL, 1], f32, kind="ExternalInput")
    bv_d = nc.dram_tensor("bv", [DL, 1], f32, kind="ExternalInput")
    O_d = nc.dram_tensor("O", [S, DM], f32, kind="ExternalOutput")

    with tile.TileContext(nc) as tc:
        with (
            tc.tile_pool(name="const", bufs=1) as const,
            tc.tile_pool(name="persist", bufs=1) as sb,
            tc.tile_pool(name="stat", bufs=6) as stat,
            tc.tile_pool(name="outp", bufs=3) as outp,
            tc.tile_pool(name="inp", bufs=2) as inp,
            tc.tile_pool(name="vtp", bufs=1) as vtp,
            tc.tile_pool(name="attp", bufs=20) as attp,
            tc.tile_pool(name="pps", bufs=2, space="PSUM") as pps,
            tc.tile_pool(name="ppx", bufs=1, space="PSUM") as ppx,
        ):
            # ---------------- constants ----------------
            # One DMA per weight: DRAM [(t p), c] -> SBUF [p, (t c)] so the
            # m-th 128-row block lands at free offset m*DL.
            def load_w(dram, nm, dt_, cols, eng):
                t = const.tile([P, MT * cols], dt_, name=nm, tag=nm)
                eng.dma_start(
                    t[:].rearrange("p (t c) -> p t c", t=MT),
                    dram.ap().rearrange("(t p) c -> p t c", p=P))
                return t

            wq_sb = load_w(wq_d, "wq", PROJ_DT, DL, nc.scalar)  # [128, 8*256]
            wk_sb = load_w(wk_d, "wk", PROJ_DT, DL, nc.scalar)
            bq_sb = const.tile([P, DPT], f32, name="bq", tag="bq")
            nc.sync.dma_start(
                bq_sb[:].rearrange("p (t c) -> p t c", t=DPT),
                bq_d.ap().rearrange("(t p) c -> p t c", p=P))
            bk_sb = const.tile([P, DPT], f32, name="bk", tag="bk")
            nc.sync.dma_start(
                bk_sb[:].rearrange("p (t c) -> p t c", t=DPT),
                bk_d.ap().rearrange("(t p) c -> p t c", p=P))
            bv_sb = const.tile([P, DPT], f32, name="bv", tag="bv")
            nc.sync.dma_start(
                bv_sb[:].rearrange("p (t c) -> p t c", t=DPT),
                bv_d.ap().rearrange("(t p) c -> p t c", p=P))
            wv_cell = []

            def load_wv():
                if not wv_cell:
                    wv_cell.append(load_w(wv_d, "wv", PROJ_DT, DL, nc.sync))
                return wv_cell[0]

            def w_slice(w, m, dp):
                return w[:, m * DL + dp * P:m * DL + (dp + 1) * P]

            wo_cell = []

            def mk_proj(it):
                return emit_projections(
                    nc, tc, it, const, sb, inp, vtp, ppx,
                    qT_in, kT_in, vT_in,
                    wq_sb, wk_sb, load_wv, bq_sb, bk_sb, bv_sb, w_slice)

            # rep 0's projections run as a plain prologue; every later
            # rep's are fed into the previous rep's attention j-loop.
            st, cl = mk_proj(0)
            pr = tc.high_priority()
            pr.__enter__()
            for c in cl:
                c()
            pr.__exit__(None, None, None)
            for it in range(reps):
                if it + 1 < reps:
                    st_next, cl_next = mk_proj(it + 1)
                else:
                    st_next, cl_next = None, []
                emit_attention(nc, tc, it, st, cl_next, const, stat, outp,
                               attp, pps, ppx, wo_d, O_d, wo_cell)
                st = st_next

    if LDW_REUSE:
        _dedupe_ldweights(nc)
    if split_waits:
        _split_excess_waits(nc)
    return nc


def _get_program():
    global _PROGRAM
    if _PROGRAM is None:
        _PROGRAM = build_program()
    return _PROGRAM


def _tf32(x):
    """Round fp32 -> TF32 (10-bit mantissa), round-to-nearest-even."""
    x = np.ascontiguousarray(np.asarray(x, dtype=np.float32))
    u = x.view(np.uint32)
    r = ((u >> 13) & 1).astype(np.uint32)
    u2 = ((u + np.uint32(0x0FFF) + r) & np.uint32(0xFFFFE000))
    return u2.view(np.float32)


def shard_inputs(inputs):
    """FULL inputs -> per-core in_maps (list of 8 dicts)."""
    q = np.asarray(inputs["query"], dtype=np.float32)
    k = np.asarray(inputs["key"], dtype=np.float32)
    v = np.asarray(inputs["value"], dtype=np.float32)
    Wq = np.asarray(inputs["Wq"], dtype=np.float32)
    Wk = np.asarray(inputs["Wk"], dtype=np.float32)
    Wv = np.asarray(inputs["Wv"], dtype=np.float32)
    Wo = np.asarray(inputs["Wo"], dtype=np.float32)
    bq = np.asarray(inputs["bq"], dtype=np.float32)
    bk = np.asarray(inputs["bk"], dtype=np.float32)
    bv = np.asarray(inputs["bv"], dtype=np.float32)
    bo = np.asarray(inputs["bo"], dtype=np.float32)

    if PROJ_BF16:
        import ml_dtypes

        def _proj_cast(x):
            return np.ascontiguousarray(np.asarray(x, np.float32)).astype(
                ml_dtypes.bfloat16)
    else:
        _proj_cast = _tf32

    qT = [_proj_cast(q[b].T) for b in range(B)]
    kT = [_proj_cast(k[b].T) for b in range(B)]
    vT = [_proj_cast(v[b].T) for b in range(B)]

    in_maps = []
    for c in range(N_CORES):
        b, g = c // GROUPS, c % GROUPS
        sl = slice(g * DL, (g + 1) * DL)
        in_maps.append({
            "qT_in": qT[b],
            "kT_in": kT[b],
            "vT_in": vT[b],
            "wq": _proj_cast(Wq[:, sl]),
            "wk": _proj_cast(Wk[:, sl]),
            "wv": _proj_cast(Wv[:, sl]),
            "wo": _proj_cast(Wo[sl, :]),
            "bq": np.ascontiguousarray(bq[sl].reshape(DL, 1)),
            "bk": np.ascontiguousarray(bk[sl].reshape(DL, 1)),
            "bv": np.ascontiguousarray(bv[sl].reshape(DL, 1)),
        })
    return in_maps


def unshard_output(results, bo):
    """results: list of 8 dicts with 'O' [S, DM] -> full [B, S, DM].
    bo is added here (host) -- cheaper than a K=1 PE matmul on device."""
    out = np.zeros((B, S, DM), np.float32)
    for c in range(N_CORES):
        out[c // GROUPS] += results[c]["O"]
    out += np.asarray(bo, np.float32)
    return out


def kernel(**inputs):
    nc = _get_program()
    in_maps = shard_inputs(inputs)
    res = run_bass_kernel_spmd(nc, in_maps, core_ids=list(range(N_CORES)))
    return unshard_output(res.results, inputs["bo"])



# revision 20
# speedup vs baseline: 1.2931x; 1.2931x over previous
"""Multi-head attention (softmax over query axis) on 8 Trainium2 cores.

Problem: nn_MultiHeadAttention_3899830305178
  B=2, S=2048, D_MODEL=1024, HEADS=16, D_K=64, fp32 IO.
  reference:
    q = (query @ Wq + bq), k = ..., v = ...        [b, s, h, dk]
    scores = einsum('bihd,bjhd->bijh', q, k) / 8
    attn = softmax(scores, axis=1)                 # over QUERY axis i (quirk)
    x = einsum('bijh,bjhd->bihd', attn, v)         [b, s, h*dk]
    out = x @ Wo + bo

Sharding: data-parallel over batch (2) x tensor-parallel over heads (4 groups
of 4 heads) = 8 cores. Each core computes a partial output
O_part = x_local @ Wo[rows of its heads]; the host sums the 4 partials per
batch (row-parallel unshard) -- bo is added on-device by the g==0 core.

Per-core kernel math (host passes query/key/value pre-transposed so the
projections contract over the model dim on partitions):
  qT[d', i] = Wq_s.T @ queryT      (d' = 4 local heads x 64 = 256)
  kT[d', j] = Wk_s.T @ keyT
  vT[d', j] = Wv_s.T @ valueT (+bv), then bf16 DMA-transpose -> v[j, d']
  per head h:  sT[j, i] = kT_h.T @ qT_h / 8  (softmax over i == free axis)
               eT = exp(sT) (bf16), rowsum via a DVE tensor_scalar accum
               v_h_scaled[j, :] = v_h[j, :] / rowsum[j]   <- softmax divisor
               xT_h[d, i] = v_h_scaled.T @ eT             (contracts over j)
  O_part[i, n] = xT.T @ Wo_s; bo is added on the host during unshard.

Engine balance (per CoreSim, 230us single-shot span): PE ~169us
(projections 41 + scores 55 + attn@V 55 + out-proj 14), ACT ~138us (the
128 [128,1024] exps are irreducible -- Exp exists only on ACT), DVE
~93us, 16KB/partition PSUM exactly full (2x scores buffers + 2x
proj/attn@V buffers). Startup: wq/wk load on the (initially idle) ACT
HWDGE queue in parallel with the q/k input chunks on SP; the q/k
projection stream is ordered q0,q1,k0,q2,q3,k1..k3 to match the ACT
engine's exp demand order; wv/wo load late; qT is tiled per input chunk
so each scores matmul gates on exactly one projected chunk.
Design choices vs the naive version:
  - sibling heads (partitions 0-63 / 64-127 of the kT/qT slices) emit
    their K=64 scores matmuls interleaved with PE tile positions
    (0,0)/(64,0), letting the PE overlap row tiles (K=64 alone half-fills
    the 128x128 array);
  - eT is bf16 (same PE rate as f32r, half the SBUF/attp footprint);
  - softmax rowsums come from a DVE tensor_scalar (2-byte fast mode,
    ~0.4us/tile) instead of the ACT accum_out (+187ns/tile on the other
    near-critical engine), computed as out=(a*1)+0 in place with
    accum_out=rowsum;
  - attn@V is split by i-half into [64,1024] psums so the first half's
    output projection overlaps the second half's attention (xT is split
    per i-half so the dependency is tile-precise); pair 1's first scores
    are emitted inside pair 0's phase B so the ACT engine never idles
    there, and the first-half output projection (with O DMAs alternating
    between the SP and ACT hardware queues) drains during pair 1's
    phase B;
  - the bias matmuls (K=1 ones-row) were removed from the PE: bo rides
    the host-side unshard sum.

Projection inputs/weights are bf16; scores and the output projection run
in float32r (TF32, fp32 accumulate) with fp32 softmax statistics; attn@V
is bf16 x bf16 -> fp32. Measured end-to-end relative error vs the fp64
reference is ~4.4e-3 on hardware.
"""

import numpy as np

import concourse.bass as bass
import concourse.mybir as mybir
import concourse.tile as tile
from concourse.bass_utils import run_bass_kernel_spmd

# problem shape (hardcoded per contract)
B, S, DM, H, DK = 2, 2048, 1024, 16, 64
N_CORES = 8
GROUPS = 4              # head groups (tensor-parallel)
HL = H // GROUPS        # 4 local heads per core
DL = HL * DK            # 256 local concat width
P = 128
SJ = S // P             # 16 strips of 128 along j (keys) and i (out rows)
MT = DM // P            # 8 contraction tiles for projections
DPT = DL // P           # 2 partition tiles of the local concat dim
SCALE = 1.0 / 8.0       # 1/sqrt(DK)

f32 = mybir.dt.float32
f32r = mybir.dt.float32r
bf16 = mybir.dt.bfloat16
AF = mybir.ActivationFunctionType

# Projection stage (inputs + projection weights) in bf16: halves the input
# DMA (the critical-path prefix) at ~2e-3 relative error. Attention and
# output projection stay TF32.
PROJ_BF16 = True
PROJ_DT = bf16 if PROJ_BF16 else f32r

import os as _os
# Reuse PE stationary weights across same-lhsT matmul runs by suppressing
# the per-matmul LDWEIGHTS (InstMatmult.ldweights=False on the trailing
# matmuls of each run).
LDW_REUSE = _os.environ.get("LDW_REUSE", "1") == "1"
# Which j parity routes its softmax rowsums to the DVE (the other parity
# uses the exp's ACT-side accumulator). 2 = all rowsums on ACT.
ROWSUM_DVE_PARITY = int(_os.environ.get("ROWSUM_DVE_PARITY", "1"))

PRIO_W = 1000000  # priority window per rep

_PROGRAM = None


def _dedupe_ldweights(nc):
    """Drop InstLdweights that reload the exact weights already resident.

    Tile's legalizer splits every matmul into (InstLdweights, InstMatmult
    ldweights=False); each reload costs ~120-180ns serialized into the PE
    stream. When consecutive LDWEIGHTS on the PE stream have identical
    weight APs (the kernel emits same-lhsT matmul runs for scores, attn@V
    and the output projection), the duplicates are pure overhead: the
    array still holds the weights (nothing else writes it), and the WAR
    protection on the SBUF region anchors on the matmuls (Tile tracked
    them as the lhsT readers), so dropping the reload is safe. Waits and
    sem updates of a dropped LDWEIGHTS move onto a NOP in its place."""
    n = 0
    for f in nc.m.functions:
        for blk in f.blocks:
            last_key = None
            new_insts = []
            for inst in blk.instructions:
                if getattr(inst, "engine", None) == mybir.EngineType.PE:
                    tn = type(inst).__name__
                    if tn == "InstLdweights":
                        key = (str(inst.ins[0]), str(inst.tile_position),
                               str(inst.perf_mode), str(inst.is_transpose))
                        if key == last_key:
                            si = inst.sync_info
                            if si is not None and (si.on_wait or si.on_update):
                                new_insts.append(mybir.InstNoOp(
                                    name=f"{inst.name}-ldwskip",
                                    engine=inst.engine,
                                    sync_info=si,
                                    bass_nofuse=True))
                            n += 1
                            continue
                        last_key = key
                    elif tn == "InstMatmult":
                        if inst.is_transpose:
                            last_key = None
                    elif tn in ("InstNoOp", "InstEventSemaphore"):
                        pass
                    else:
                        last_key = None
                new_insts.append(inst)
            blk.instructions[:] = new_insts
    return n


def _split_excess_waits(nc, max_waits=1):
    """walrus in this container rejects >1 semaphore wait per instruction
    (e.g. the Tile kernel-tail Drain); move extras onto same-engine NOPs."""
    n_split = 0
    for f in nc.m.functions:
        for blk in f.blocks:
            new_insts = []
            for inst in blk.instructions:
                si = getattr(inst, "sync_info", None)
                if si is not None and si.on_wait and len(si.on_wait) > max_waits:
                    waits = list(si.on_wait)
                    extra, keep = waits[:-max_waits], waits[-max_waits:]
                    for i in range(0, len(extra), max_waits):
                        chunk = extra[i:i + max_waits]
                        nop = mybir.InstNoOp(
                            name=f"{inst.name}-ws{n_split}-{i}",
                            engine=inst.engine,
                            sync_info=mybir.SyncInfo(on_wait=chunk, on_update=[]),
                            bass_nofuse=True,
                        )
                        new_insts.append(nop)
                    si.on_wait = keep
                    n_split += 1
                new_insts.append(inst)
            blk.instructions[:] = new_insts
    return n_split


QK_ORDER = [("q", 0), ("k", 0), ("q", 1), ("q", 2), ("q", 3),
            ("k", 1), ("k", 2), ("k", 3)]


def emit_projections(nc, tc, it, const, sb, inp, vtp, ppx,
                     qT_in, kT_in, vT_in,
                     wq_sb, wk_sb, load_wv, bq_sb, bk_sb, bv_sb, w_slice):
    """Create rep-it's activation tiles and return (state, chunk closures).

    Each closure emits one input chunk's DMA + projection matmuls + bias
    adds (and for v, the v4 transposes). The caller interleaves the
    closures of rep it+1 into rep it's attention j-loop so the static
    scheduler places the projection matmuls inside the attention span
    (the PE has ~2x headroom there); without this the scheduler abuts the
    reps and the ACT stream stalls ~50-60us per rep waiting for
    projections."""
    R = f"_r{it}"
    # bufs=2 on qT/kT/v4: the next rep's projections write the other
    # buffer while this rep's attention still reads this one.
    qT_sb = [[sb.tile([P, 512], bf16, name=f"qT{dp}_{i4}{R}",
                      tag=f"qT{dp}_{i4}", bufs=2) for i4 in range(4)]
             for dp in range(DPT)]
    kT_sb = [[sb.tile([P, 512], bf16, name=f"kT{dp}_{jg}{R}",
                      tag=f"kT{dp}_{jg}", bufs=2) for jg in range(4)]
             for dp in range(DPT)]
    # v packed per j-group of 4: v4_sb[jg][p, jj*DL + d'] holds
    # v[jg*512 + jj*128 + p, d']
    v4_sb = [sb.tile([P, 4 * DL], bf16, name=f"v{jg}{R}", tag=f"v{jg}",
                     bufs=2)
             for jg in range(4)]
    xT_sb = [[sb.tile([P, 1024], bf16, name=f"xT{hp}_{ih}{R}",
                      tag=f"xT{hp}_{ih}") for ih in range(2)]
             for hp in range(DPT)]
    vT_sb = [vtp.tile([P, S], bf16, name=f"vT{dp}{R}", tag=f"vT{dp}")
             for dp in range(DPT)]

    def load_in_chunk(win, nm, i4):
        # one DMA: all 8 m-blocks of columns [i0, i0+512)
        t = inp.tile([P, MT * 512], PROJ_DT, name=f"{nm}in{i4}{R}",
                     tag="pin")
        src = win.ap().rearrange("(t p) c -> p t c", p=P)
        nc.sync.dma_start(
            t[:].rearrange("p (t c) -> p t c", t=MT),
            src[:, :, i4 * 512:(i4 + 1) * 512])
        return t

    def qk_chunk(nm, i4):
        def emit():
            win, w_sb, b_sb = ((qT_in, wq_sb, bq_sb) if nm == "q"
                               else (kT_in, wk_sb, bk_sb))
            ch = load_in_chunk(win, nm, i4)
            for dp in range(DPT):
                ps = ppx.tile([P, 512], f32, name=f"ps{nm}{i4}_{dp}{R}",
                              tag="px", bufs=2)
                for m in range(MT):
                    nc.tensor.matmul(
                        ps[:], w_slice(w_sb, m, dp),
                        ch[:, m * 512:(m + 1) * 512],
                        start=(m == 0), stop=(m == MT - 1))
                dst = (qT_sb if nm == "q" else kT_sb)[dp][i4][:]
                nc.vector.tensor_scalar_add(dst, ps[:], b_sb[:, dp:dp + 1])
        return emit

    def v_chunk(i4):
        def emit():
            wv_sb = load_wv()
            i0 = i4 * 512
            ch = load_in_chunk(vT_in, "v", i4)
            for dp in range(DPT):
                ps = ppx.tile([P, 512], f32, name=f"psvt{i4}_{dp}{R}",
                              tag="px", bufs=2)
                for m in range(MT):
                    nc.tensor.matmul(
                        ps[:], w_slice(wv_sb, m, dp),
                        ch[:, m * 512:(m + 1) * 512],
                        start=(m == 0), stop=(m == MT - 1))
                nc.vector.tensor_scalar_add(
                    vT_sb[dp][:, i0:i0 + 512], ps[:], bv_sb[:, dp:dp + 1])
            for dp in range(DPT):
                out_view = v4_sb[i4][:].rearrange(
                    "p (j c) -> p j c", j=4)[:, :, dp * P:(dp + 1) * P]
                nc.sync.dma_start(
                    out_view, vT_sb[dp][:, i0:i0 + 512], transpose=True)
        return emit

    closures = [qk_chunk(nm, i4) for nm, i4 in QK_ORDER]
    closures += [v_chunk(i4) for i4 in range(4)]
    st = dict(qT_sb=qT_sb, kT_sb=kT_sb, v4_sb=v4_sb, xT_sb=xT_sb)
    return st, closures


def emit_attention(nc, tc, it, st, const, stat, outp, attp, pps,
                   ppx, wo_d, O_d, wo_cell):
    """Attention + output projection for rep it; `feeder` holds the next
    rep's projection-chunk closures, interleaved into the j-loop."""
    R = f"_r{it}"
    qT_sb, kT_sb = st["qT_sb"], st["kT_sb"]
    v4_sb, xT_sb = st["v4_sb"], st["xT_sb"]

    # ---------------- attention ----------------
    # Heads run in sibling pairs (2hp, 2hp+1) whose kT/qT slices live at
    # partitions 0-63 / 64-127 (PE row tiles 0/64). Per (head, j) the four
    # scores matmuls (2 i-halves x 2 i-chunks) share one kT stationary
    # slice and the four attn@V matmuls share one vsc slice: with
    # LDW_REUSE the trailing matmuls set InstMatmult.ldweights=False so
    # walrus skips the per-matmul LDWEIGHTS reload (HW trace showed the
    # 768 reloads/rep serialize ~130ns each into the PE stream).
    # Softmax rowsums ride the exp's ACT-side accumulator on even j and a
    # DVE tensor_scalar on odd j, balancing the two near-critical engines.
    # attn@V accumulates both i-halves into two [128, 1024] psums held for
    # the whole pair (sibling heads at psum partitions 0-63/64-127).

    MULT = mybir.AluOpType.mult
    ADD = mybir.AluOpType.add

    def head_scores(hp, hh, j):
        """scores + exp + rowsum for one head, full i range (2 psum tiles)."""
        jg, jr = divmod(j, 4)
        h = hp * 2 + hh
        base = hh * 64
        lhs = kT_sb[hp][jg][base:base + 64, jr * P:(jr + 1) * P]
        pss = []
        first = True
        for ih in range(2):
            ps = pps.tile([P, 1024], f32, name=f"ps{h}_{j}_{ih}{R}",
                          tag="ps")
            for i5 in range(2):
                mm = nc.tensor.matmul(
                    ps[:, i5 * 512:(i5 + 1) * 512], lhs,
                    qT_sb[hp][ih * 2 + i5][base:base + 64, :],
                    start=True, stop=True)
                if LDW_REUSE and not first:
                    mm.ins.ldweights = False
                first = False
            pss.append(ps)
        outs = []
        for ih, ps in enumerate(pss):
            a = attp.tile([P, 1024], bf16, name=f"att{h}_{j}_{ih}{R}",
                          tag=f"att{ih}", bufs=(12 if ih == 0 else 33))
            rsh = stat.tile([P, 1], f32, name=f"rsh{h}_{j}_{ih}{R}",
                            tag="rsh", bufs=16)
            if j % 2 == ROWSUM_DVE_PARITY:
                nc.scalar.activation(a[:], ps[:], AF.Exp, scale=SCALE)
                nc.vector.tensor_scalar(a[:], a[:], 1.0, 0.0, MULT, ADD,
                                        accum_out=rsh[:])
            else:
                nc.scalar.activation(a[:], ps[:], AF.Exp, scale=SCALE,
                                     accum_out=rsh[:])
            outs.append((a, rsh))
        return outs

    def head_finish_a(hp, hh, j, xph, outs):
        """softmax divisor onto v, then attn@V for i-half 0; saves the
        i-half-1 eT and the vsc tile for phase B."""
        h = hp * 2 + hh
        jg, jr = divmod(j, 4)
        rs = stat.tile([P, 1], f32, name=f"rs{h}_{j}{R}", tag="rs")
        nc.vector.tensor_add(rs[:], outs[0][1][:], outs[1][1][:])
        rc = stat.tile([P, 1], f32, name=f"rc{h}_{j}{R}", tag="rc")
        nc.vector.reciprocal(rc[:], rs[:])
        vsc = attp.tile([P, 64], bf16, name=f"vsc{h}_{j}{R}", tag="vsc",
                        bufs=34)
        nc.vector.tensor_scalar_mul(
            vsc[:],
            v4_sb[jg][:, jr * DL + h * 64:jr * DL + (h + 1) * 64],
            rc[:])
        vsc_t[h][j] = vsc
        a1_t[h][j] = outs[1][0]
        for i5 in range(2):
            nc.tensor.matmul(
                xph[hh * 64:(hh + 1) * 64, i5 * 512:(i5 + 1) * 512],
                vsc[:], outs[0][0][:, i5 * 512:(i5 + 1) * 512],
                start=(j == 0), stop=(j == SJ - 1),
                skip_group_check=True)

    def head_finish_b(hp, hh, j, xph):
        """attn@V for i-half 1 from the saved eT/vsc (pure PE work)."""
        h = hp * 2 + hh
        a = a1_t[h][j]
        for i5 in range(2):
            nc.tensor.matmul(
                xph[hh * 64:(hh + 1) * 64, i5 * 512:(i5 + 1) * 512],
                vsc_t[h][j][:], a[:, i5 * 512:(i5 + 1) * 512],
                start=(j == 0), stop=(j == SJ - 1),
                skip_group_check=True)

    vsc_t = [[None] * SJ for _ in range(HL)]
    a1_t = [[None] * SJ for _ in range(HL)]

    # ---------------- output projection constants ----------------
    # bo is added on the host during unshard (a K=1 ones-row matmul for it
    # here would cost 16384 PE rows ~ 7us).
    if not wo_cell:
        wo_sb = const.tile([P, DPT * DM], bf16, name="wo", tag="wo")
        nc.sync.dma_start(
            wo_sb[:].rearrange("p (t c) -> p t c", t=DPT),
            wo_d.ap().rearrange("(t p) c -> p t c", p=P))
        wo_cell.append(wo_sb)
    wo_sb = wo_cell[0]

    def emit_outproj(jts):
        for jt in jts:
            ot = outp.tile([P, DM], f32, name=f"ot{jt}{R}", tag="ot")
            ps = ppx.tile([P, DM], f32, name=f"pso{jt}{R}", tag="xps",
                          bufs=1)
            jh, jo = divmod(jt, 8)
            for cpt in range(DPT):
                first = True
                for n5 in range(2):
                    no = n5 * 512
                    mm = nc.tensor.matmul(
                        ps[:, no:no + 512],
                        xT_sb[cpt][jh][:, jo * P:(jo + 1) * P],
                        wo_sb[:, cpt * DM + no:cpt * DM + no + 512],
                        start=(cpt == 0), stop=(cpt == DPT - 1))
                    if LDW_REUSE and not first:
                        mm.ins.ldweights = False
                    first = False
            nc.vector.tensor_copy(ot[:], ps[:])
            oq = nc.sync if jt % 2 == 0 else nc.gpsimd
            oq.dma_start(O_d.ap()[jt * P:(jt + 1) * P, :], ot[:])

    def alloc_xps(hp, ih):
        return ppx.tile([P, 1024], f32, name=f"xp{ih}_{hp}{R}", tag="xps",
                        bufs=1)

    def copy_xps(hp, ih, xph):
        nc.vector.tensor_copy(xT_sb[hp][ih][:], xph[:])

    # Priority bands (lower = scheduled earlier among READY instructions).
    # The Tile list-scheduler fills each engine's idle ticks with the
    # lowest-priority ready work, so: the exp-feeding lane (scores/exp/
    # rowsum/attn@V-A) owns the rep's band; phase B (pure-PE attn@V from
    # saved tiles) fills its gaps; the PREVIOUS rep's phase-B/outproj and
    # the NEXT rep's projections (emitted in band base+0 by the caller)
    # fill what's left. Dependencies rate-limit everything, so a filler
    # can never stall the lane beyond one instruction.
    base = it * PRIO_W
    tc.cur_priority = base + 10000
    hp = 0
    xph = alloc_xps(hp, 0)
    for j in range(SJ):
        oA = head_scores(hp, 0, j)
        oB = head_scores(hp, 1, j)
        head_finish_a(hp, 0, j, xph, oA)
        head_finish_a(hp, 1, j, xph, oB)
    copy_xps(hp, 0, xph)
    tc.cur_priority = base + 16000
    xph = alloc_xps(hp, 1)
    for j in range(SJ):
        head_finish_b(hp, 0, j, xph)
        head_finish_b(hp, 1, j, xph)
    copy_xps(hp, 1, xph)

    hp = 1
    tc.cur_priority = base + 13000
    xph = alloc_xps(hp, 0)
    for j in range(SJ):
        oA = head_scores(hp, 0, j)
        oB = head_scores(hp, 1, j)
        head_finish_a(hp, 0, j, xph, oA)
        head_finish_a(hp, 1, j, xph, oB)
    copy_xps(hp, 0, xph)
    tc.cur_priority = base + 117500
    xph = alloc_xps(hp, 1)
    for j in range(SJ):
        head_finish_b(hp, 0, j, xph)
        head_finish_b(hp, 1, j, xph)
    copy_xps(hp, 1, xph)

    tc.cur_priority = base + 118500
    # output projection: filler band, overlaps the next rep's attention
    emit_outproj(range(SJ))


def build_program(split_waits=True, reps=1):
    nc = bass.Bass("TRN2", target_bir_lowering=False, debug=False)

    qT_in = nc.dram_tensor("qT_in", [DM, S], PROJ_DT, kind="ExternalInput")
    kT_in = nc.dram_tensor("kT_in", [DM, S], PROJ_DT, kind="ExternalInput")
    vT_in = nc.dram_tensor("vT_in", [DM, S], PROJ_DT, kind="ExternalInput")
    wq_d = nc.dram_tensor("wq", [DM, DL], PROJ_DT, kind="ExternalInput")
    wk_d = nc.dram_tensor("wk", [DM, DL], PROJ_DT, kind="ExternalInput")
    wv_d = nc.dram_tensor("wv", [DM, DL], PROJ_DT, kind="ExternalInput")
    wo_d = nc.dram_tensor("wo", [DL, DM], bf16, kind="ExternalInput")
    bq_d = nc.dram_tensor("bq", [DL, 1], f32, kind="ExternalInput")
    bk_d = nc.dram_tensor("bk", [DL, 1], f32, kind="ExternalInput")
    bv_d = nc.dram_tensor("bv", [DL, 1], f32, kind="ExternalInput")
    O_d = nc.dram_tensor("O", [S, DM], f32, kind="ExternalOutput")

    with tile.TileContext(nc) as tc:
        with (
            tc.tile_pool(name="const", bufs=1) as const,
            tc.tile_pool(name="persist", bufs=1) as sb,
            tc.tile_pool(name="stat", bufs=6) as stat,
            tc.tile_pool(name="outp", bufs=3) as outp,
            tc.tile_pool(name="inp", bufs=2) as inp,
            tc.tile_pool(name="vtp", bufs=1) as vtp,
            tc.tile_pool(name="attp", bufs=20) as attp,
            tc.tile_pool(name="pps", bufs=2, space="PSUM") as pps,
            tc.tile_pool(name="ppx", bufs=1, space="PSUM") as ppx,
        ):
            # ---------------- constants ----------------
            # One DMA per weight: DRAM [(t p), c] -> SBUF [p, (t c)] so the
            # m-th 128-row block lands at free offset m*DL.
            def load_w(dram, nm, dt_, cols, eng):
                t = const.tile([P, MT * cols], dt_, name=nm, tag=nm)
                eng.dma_start(
                    t[:].rearrange("p (t c) -> p t c", t=MT),
                    dram.ap().rearrange("(t p) c -> p t c", p=P))
                return t

            wq_sb = load_w(wq_d, "wq", PROJ_DT, DL, nc.scalar)  # [128, 8*256]
            wk_sb = load_w(wk_d, "wk", PROJ_DT, DL, nc.scalar)
            bq_sb = const.tile([P, DPT], f32, name="bq", tag="bq")
            nc.sync.dma_start(
                bq_sb[:].rearrange("p (t c) -> p t c", t=DPT),
                bq_d.ap().rearrange("(t p) c -> p t c", p=P))
            bk_sb = const.tile([P, DPT], f32, name="bk", tag="bk")
            nc.sync.dma_start(
                bk_sb[:].rearrange("p (t c) -> p t c", t=DPT),
                bk_d.ap().rearrange("(t p) c -> p t c", p=P))
            bv_sb = const.tile([P, DPT], f32, name="bv", tag="bv")
            nc.sync.dma_start(
                bv_sb[:].rearrange("p (t c) -> p t c", t=DPT),
                bv_d.ap().rearrange("(t p) c -> p t c", p=P))
            wv_cell = []

            def load_wv():
                if not wv_cell:
                    wv_cell.append(load_w(wv_d, "wv", PROJ_DT, DL, nc.sync))
                return wv_cell[0]

            def w_slice(w, m, dp):
                return w[:, m * DL + dp * P:m * DL + (dp + 1) * P]

            wo_cell = []

            def mk_proj(it):
                return emit_projections(
                    nc, tc, it, const, sb, inp, vtp, ppx,
                    qT_in, kT_in, vT_in,
                    wq_sb, wk_sb, load_wv, bq_sb, bk_sb, bv_sb, w_slice)

            # rep 0's projections are the prologue; rep it+1's are
            # emitted in band (it+1)*PRIO_W + 0 so the scheduler hoists
            # them into rep it's attention as soon as their DMAs/slots
            # allow (the qT/kT/v4 double-buffering makes that legal).
            st, cl = mk_proj(0)
            tc.cur_priority = 0
            for c in cl:
                c()
            for it in range(reps):
                if it + 1 < reps:
                    st_next, cl_next = mk_proj(it + 1)
                    tc.cur_priority = (it + 1) * PRIO_W
                    for c in cl_next:
                        c()
                else:
                    st_next = None
                emit_attention(nc, tc, it, st, const, stat, outp,
                               attp, pps, ppx, wo_d, O_d, wo_cell)
                st = st_next

    if LDW_REUSE:
        _dedupe_ldweights(nc)
    if split_waits:
        _split_excess_waits(nc)
    return nc


def _get_program():
    global _PROGRAM
    if _PROGRAM is None:
        _PROGRAM = build_program()
    return _PROGRAM


def _tf32(x):
    """Round fp32 -> TF32 (10-bit mantissa), round-to-nearest-even."""
    x = np.ascontiguousarray(np.asarray(x, dtype=np.float32))
    u = x.view(np.uint32)
    r = ((u >> 13) & 1).astype(np.uint32)
    u2 = ((u + np.uint32(0x0FFF) + r) & np.uint32(0xFFFFE000))
    return u2.view(np.float32)


def shard_inputs(inputs):
    """FULL inputs -> per-core in_maps (list of 8 dicts)."""
    q = np.asarray(inputs["query"], dtype=np.float32)
    k = np.asarray(inputs["key"], dtype=np.float32)
    v = np.asarray(inputs["value"], dtype=np.float32)
    Wq = np.asarray(inputs["Wq"], dtype=np.float32)
    Wk = np.asarray(inputs["Wk"], dtype=np.float32)
    Wv = np.asarray(inputs["Wv"], dtype=np.float32)
    Wo = np.asarray(inputs["Wo"], dtype=np.float32)
    bq = np.asarray(inputs["bq"], dtype=np.float32)
    bk = np.asarray(inputs["bk"], dtype=np.float32)
    bv = np.asarray(inputs["bv"], dtype=np.float32)
    bo = np.asarray(inputs["bo"], dtype=np.float32)

    if PROJ_BF16:
        import ml_dtypes

        def _proj_cast(x):
            return np.ascontiguousarray(np.asarray(x, np.float32)).astype(
                ml_dtypes.bfloat16)
    else:
        _proj_cast = _tf32

    qT = [_proj_cast(q[b].T) for b in range(B)]
    kT = [_proj_cast(k[b].T) for b in range(B)]
    vT = [_proj_cast(v[b].T) for b in range(B)]

    in_maps = []
    for c in range(N_CORES):
        b, g = c // GROUPS, c % GROUPS
        sl = slice(g * DL, (g + 1) * DL)
        in_maps.append({
            "qT_in": qT[b],
            "kT_in": kT[b],
            "vT_in": vT[b],
            "wq": _proj_cast(Wq[:, sl]),
            "wk": _proj_cast(Wk[:, sl]),
            "wv": _proj_cast(Wv[:, sl]),
            "wo": _proj_cast(Wo[sl, :]),
            "bq": np.ascontiguousarray(bq[sl].reshape(DL, 1)),
            "bk": np.ascontiguousarray(bk[sl].reshape(DL, 1)),
            "bv": np.ascontiguousarray(bv[sl].reshape(DL, 1)),
        })
    return in_maps


def unshard_output(results, bo):
    """results: list of 8 dicts with 'O' [S, DM] -> full [B, S, DM].
    bo is added here (host) -- cheaper than a K=1 PE matmul on device."""
    out = np.zeros((B, S, DM), np.float32)
    for c in range(N_CORES):
        out[c // GROUPS] += results[c]["O"]
    out += np.asarray(bo, np.float32)
    return out


def kernel(**inputs):
    nc = _get_program()
    in_maps = shard_inputs(inputs)
    res = run_bass_kernel_spmd(nc, in_maps, core_ids=list(range(N_CORES)))
    return unshard_output(res.results, inputs["bo"])



# revision 22
# speedup vs baseline: 1.3316x; 1.0298x over previous
"""Multi-head attention (softmax over query axis) on 8 Trainium2 cores.

Problem: nn_MultiHeadAttention_3899830305178
  B=2, S=2048, D_MODEL=1024, HEADS=16, D_K=64, fp32 IO.
  reference:
    q = (query @ Wq + bq), k = ..., v = ...        [b, s, h, dk]
    scores = einsum('bihd,bjhd->bijh', q, k) / 8
    attn = softmax(scores, axis=1)                 # over QUERY axis i (quirk)
    x = einsum('bijh,bjhd->bihd', attn, v)         [b, s, h*dk]
    out = x @ Wo + bo

Sharding: data-parallel over batch (2) x tensor-parallel over heads (4 groups
of 4 heads) = 8 cores. Each core computes a partial output
O_part = x_local @ Wo[rows of its heads]; the host sums the 4 partials per
batch (row-parallel unshard) -- bo is added on-device by the g==0 core.

Per-core kernel math (host passes query/key/value pre-transposed so the
projections contract over the model dim on partitions):
  qT[d', i] = Wq_s.T @ queryT      (d' = 4 local heads x 64 = 256)
  kT[d', j] = Wk_s.T @ keyT
  vT[d', j] = Wv_s.T @ valueT (+bv), then bf16 DMA-transpose -> v[j, d']
  per head h:  sT[j, i] = kT_h.T @ qT_h / 8  (softmax over i == free axis)
               eT = exp(sT) (bf16), rowsum via a DVE tensor_scalar accum
               v_h_scaled[j, :] = v_h[j, :] / rowsum[j]   <- softmax divisor
               xT_h[d, i] = v_h_scaled.T @ eT             (contracts over j)
  O_part[i, n] = xT.T @ Wo_s; bo is added on the host during unshard.

Engine balance (per CoreSim, 230us single-shot span): PE ~169us
(projections 41 + scores 55 + attn@V 55 + out-proj 14), ACT ~138us (the
128 [128,1024] exps are irreducible -- Exp exists only on ACT), DVE
~93us, 16KB/partition PSUM exactly full (2x scores buffers + 2x
proj/attn@V buffers). Startup: wq/wk load on the (initially idle) ACT
HWDGE queue in parallel with the q/k input chunks on SP; the q/k
projection stream is ordered q0,q1,k0,q2,q3,k1..k3 to match the ACT
engine's exp demand order; wv/wo load late; qT is tiled per input chunk
so each scores matmul gates on exactly one projected chunk.
Design choices vs the naive version:
  - sibling heads (partitions 0-63 / 64-127 of the kT/qT slices) emit
    their K=64 scores matmuls interleaved with PE tile positions
    (0,0)/(64,0), letting the PE overlap row tiles (K=64 alone half-fills
    the 128x128 array);
  - eT is bf16 (same PE rate as f32r, half the SBUF/attp footprint);
  - softmax rowsums come from a DVE tensor_scalar (2-byte fast mode,
    ~0.4us/tile) instead of the ACT accum_out (+187ns/tile on the other
    near-critical engine), computed as out=(a*1)+0 in place with
    accum_out=rowsum;
  - attn@V is split by i-half into [64,1024] psums so the first half's
    output projection overlaps the second half's attention (xT is split
    per i-half so the dependency is tile-precise); pair 1's first scores
    are emitted inside pair 0's phase B so the ACT engine never idles
    there, and the first-half output projection (with O DMAs alternating
    between the SP and ACT hardware queues) drains during pair 1's
    phase B;
  - the bias matmuls (K=1 ones-row) were removed from the PE: bo rides
    the host-side unshard sum.

Projection inputs/weights are bf16; scores and the output projection run
in float32r (TF32, fp32 accumulate) with fp32 softmax statistics; attn@V
is bf16 x bf16 -> fp32. Measured end-to-end relative error vs the fp64
reference is ~4.4e-3 on hardware.
"""

import numpy as np

import concourse.bass as bass
import concourse.mybir as mybir
import concourse.tile as tile
from concourse.bass_utils import run_bass_kernel_spmd

# problem shape (hardcoded per contract)
B, S, DM, H, DK = 2, 2048, 1024, 16, 64
N_CORES = 8
GROUPS = 4              # head groups (tensor-parallel)
HL = H // GROUPS        # 4 local heads per core
DL = HL * DK            # 256 local concat width
P = 128
SJ = S // P             # 16 strips of 128 along j (keys) and i (out rows)
MT = DM // P            # 8 contraction tiles for projections
DPT = DL // P           # 2 partition tiles of the local concat dim
SCALE = 1.0 / 8.0       # 1/sqrt(DK)

f32 = mybir.dt.float32
f32r = mybir.dt.float32r
bf16 = mybir.dt.bfloat16
AF = mybir.ActivationFunctionType

# Projection stage (inputs + projection weights) in bf16: halves the input
# DMA (the critical-path prefix) at ~2e-3 relative error. Attention and
# output projection stay TF32.
PROJ_BF16 = True
PROJ_DT = bf16 if PROJ_BF16 else f32r

import os as _os
# Reuse PE stationary weights across same-lhsT matmul runs by suppressing
# the per-matmul LDWEIGHTS (InstMatmult.ldweights=False on the trailing
# matmuls of each run).
LDW_REUSE = _os.environ.get("LDW_REUSE", "1") == "1"
# Which j parity routes its softmax rowsums to the DVE (the other parity
# uses the exp's ACT-side accumulator). 2 = all rowsums on ACT.
ROWSUM_DVE_PARITY = int(_os.environ.get("ROWSUM_DVE_PARITY", "1"))

PRIO_W = 1000000  # priority window per rep

_PROGRAM = None


def _dedupe_ldweights(nc):
    """Drop InstLdweights that reload the exact weights already resident.

    Tile's legalizer splits every matmul into (InstLdweights, InstMatmult
    ldweights=False); each reload costs ~120-180ns serialized into the PE
    stream. When consecutive LDWEIGHTS on the PE stream have identical
    weight APs (the kernel emits same-lhsT matmul runs for scores, attn@V
    and the output projection), the duplicates are pure overhead: the
    array still holds the weights (nothing else writes it), and the WAR
    protection on the SBUF region anchors on the matmuls (Tile tracked
    them as the lhsT readers), so dropping the reload is safe. Waits and
    sem updates of a dropped LDWEIGHTS move onto a NOP in its place."""
    n = 0
    for f in nc.m.functions:
        for blk in f.blocks:
            last_key = None
            new_insts = []
            for inst in blk.instructions:
                if getattr(inst, "engine", None) == mybir.EngineType.PE:
                    tn = type(inst).__name__
                    if tn == "InstLdweights":
                        key = (str(inst.ins[0]), str(inst.tile_position),
                               str(inst.perf_mode), str(inst.is_transpose))
                        if key == last_key:
                            si = inst.sync_info
                            if si is not None and (si.on_wait or si.on_update):
                                new_insts.append(mybir.InstNoOp(
                                    name=f"{inst.name}-ldwskip",
                                    engine=inst.engine,
                                    sync_info=si,
                                    bass_nofuse=True))
                            n += 1
                            continue
                        last_key = key
                    elif tn == "InstMatmult":
                        if inst.is_transpose:
                            last_key = None
                    elif tn in ("InstNoOp", "InstEventSemaphore"):
                        pass
                    else:
                        last_key = None
                new_insts.append(inst)
            blk.instructions[:] = new_insts
    return n


def _split_excess_waits(nc, max_waits=1):
    """walrus in this container rejects >1 semaphore wait per instruction
    (e.g. the Tile kernel-tail Drain); move extras onto same-engine NOPs."""
    n_split = 0
    for f in nc.m.functions:
        for blk in f.blocks:
            new_insts = []
            for inst in blk.instructions:
                si = getattr(inst, "sync_info", None)
                if si is not None and si.on_wait and len(si.on_wait) > max_waits:
                    waits = list(si.on_wait)
                    extra, keep = waits[:-max_waits], waits[-max_waits:]
                    for i in range(0, len(extra), max_waits):
                        chunk = extra[i:i + max_waits]
                        nop = mybir.InstNoOp(
                            name=f"{inst.name}-ws{n_split}-{i}",
                            engine=inst.engine,
                            sync_info=mybir.SyncInfo(on_wait=chunk, on_update=[]),
                            bass_nofuse=True,
                        )
                        new_insts.append(nop)
                    si.on_wait = keep
                    n_split += 1
                new_insts.append(inst)
            blk.instructions[:] = new_insts
    return n_split


QK_ORDER = [("q", 0), ("k", 0), ("q", 1), ("q", 2), ("q", 3),
            ("k", 1), ("k", 2), ("k", 3)]


def emit_projections(nc, tc, it, const, sb, inp, vtp, ppx,
                     qT_in, kT_in, vT_in,
                     wq_sb, wk_sb, load_wv, bq_sb, bk_sb, bv_sb, w_slice):
    """Create rep-it's activation tiles and return (state, chunk closures).

    Each closure emits one input chunk's DMA + projection matmuls + bias
    adds (and for v, the v4 transposes). The caller interleaves the
    closures of rep it+1 into rep it's attention j-loop so the static
    scheduler places the projection matmuls inside the attention span
    (the PE has ~2x headroom there); without this the scheduler abuts the
    reps and the ACT stream stalls ~50-60us per rep waiting for
    projections."""
    R = f"_r{it}"
    # bufs=2 on qT/kT/v4: the next rep's projections write the other
    # buffer while this rep's attention still reads this one.
    qT_sb = [[sb.tile([P, 512], bf16, name=f"qT{dp}_{i4}{R}",
                      tag=f"qT{dp}_{i4}", bufs=2) for i4 in range(4)]
             for dp in range(DPT)]
    kT_sb = [[sb.tile([P, 512], bf16, name=f"kT{dp}_{jg}{R}",
                      tag=f"kT{dp}_{jg}", bufs=2) for jg in range(4)]
             for dp in range(DPT)]
    # v packed per j-group of 4: v4_sb[jg][p, jj*DL + d'] holds
    # v[jg*512 + jj*128 + p, d']
    v4_sb = [sb.tile([P, 4 * DL], bf16, name=f"v{jg}{R}", tag=f"v{jg}",
                     bufs=2)
             for jg in range(4)]
    xT_sb = [[sb.tile([P, 1024], bf16, name=f"xT{hp}_{ih}{R}",
                      tag=f"xT{hp}_{ih}") for ih in range(2)]
             for hp in range(DPT)]
    vT_sb = [vtp.tile([P, S], bf16, name=f"vT{dp}{R}", tag=f"vT{dp}")
             for dp in range(DPT)]

    def load_in_chunk(win, nm, i4):
        # one DMA: all 8 m-blocks of columns [i0, i0+512)
        t = inp.tile([P, MT * 512], PROJ_DT, name=f"{nm}in{i4}{R}",
                     tag="pin")
        src = win.ap().rearrange("(t p) c -> p t c", p=P)
        nc.sync.dma_start(
            t[:].rearrange("p (t c) -> p t c", t=MT),
            src[:, :, i4 * 512:(i4 + 1) * 512])
        return t

    def qk_chunk(nm, i4):
        def emit():
            win, w_sb, b_sb = ((qT_in, wq_sb, bq_sb) if nm == "q"
                               else (kT_in, wk_sb, bk_sb))
            ch = load_in_chunk(win, nm, i4)
            for dp in range(DPT):
                ps = ppx.tile([P, 512], f32, name=f"ps{nm}{i4}_{dp}{R}",
                              tag="px", bufs=2)
                for m in range(MT):
                    nc.tensor.matmul(
                        ps[:], w_slice(w_sb, m, dp),
                        ch[:, m * 512:(m + 1) * 512],
                        start=(m == 0), stop=(m == MT - 1))
                dst = (qT_sb if nm == "q" else kT_sb)[dp][i4][:]
                nc.vector.tensor_scalar_add(dst, ps[:], b_sb[:, dp:dp + 1])
        return emit

    def v_chunk(i4):
        def emit():
            wv_sb = load_wv()
            i0 = i4 * 512
            ch = load_in_chunk(vT_in, "v", i4)
            for dp in range(DPT):
                ps = ppx.tile([P, 512], f32, name=f"psvt{i4}_{dp}{R}",
                              tag="px", bufs=2)
                for m in range(MT):
                    nc.tensor.matmul(
                        ps[:], w_slice(wv_sb, m, dp),
                        ch[:, m * 512:(m + 1) * 512],
                        start=(m == 0), stop=(m == MT - 1))
                nc.vector.tensor_scalar_add(
                    vT_sb[dp][:, i0:i0 + 512], ps[:], bv_sb[:, dp:dp + 1])
            for dp in range(DPT):
                out_view = v4_sb[i4][:].rearrange(
                    "p (j c) -> p j c", j=4)[:, :, dp * P:(dp + 1) * P]
                nc.sync.dma_start(
                    out_view, vT_sb[dp][:, i0:i0 + 512], transpose=True)
        return emit

    closures = [qk_chunk(nm, i4) for nm, i4 in QK_ORDER]
    closures += [v_chunk(i4) for i4 in range(4)]
    st = dict(qT_sb=qT_sb, kT_sb=kT_sb, v4_sb=v4_sb, xT_sb=xT_sb)
    return st, closures


def emit_attention(nc, tc, it, st, const, stat, outp, attp, pps,
                   ppx, wo_d, O_d, wo_cell):
    """Attention + output projection for rep it; `feeder` holds the next
    rep's projection-chunk closures, interleaved into the j-loop."""
    R = f"_r{it}"
    qT_sb, kT_sb = st["qT_sb"], st["kT_sb"]
    v4_sb, xT_sb = st["v4_sb"], st["xT_sb"]

    # ---------------- attention ----------------
    # Heads run in sibling pairs (2hp, 2hp+1) whose kT/qT slices live at
    # partitions 0-63 / 64-127 (PE row tiles 0/64). Per (head, j) the four
    # scores matmuls (2 i-halves x 2 i-chunks) share one kT stationary
    # slice and the four attn@V matmuls share one vsc slice: with
    # LDW_REUSE the trailing matmuls set InstMatmult.ldweights=False so
    # walrus skips the per-matmul LDWEIGHTS reload (HW trace showed the
    # 768 reloads/rep serialize ~130ns each into the PE stream).
    # Softmax rowsums ride the exp's ACT-side accumulator on even j and a
    # DVE tensor_scalar on odd j, balancing the two near-critical engines.
    # attn@V accumulates both i-halves into two [128, 1024] psums held for
    # the whole pair (sibling heads at psum partitions 0-63/64-127).

    MULT = mybir.AluOpType.mult
    ADD = mybir.AluOpType.add

    def head_scores(hp, hh, j):
        """scores + exp + rowsum for one head, full i range (2 psum tiles)."""
        jg, jr = divmod(j, 4)
        h = hp * 2 + hh
        base = hh * 64
        lhs = kT_sb[hp][jg][base:base + 64, jr * P:(jr + 1) * P]
        pss = []
        first = True
        for ih in range(2):
            ps = pps.tile([P, 1024], f32, name=f"ps{h}_{j}_{ih}{R}",
                          tag="ps")
            for i5 in range(2):
                mm = nc.tensor.matmul(
                    ps[:, i5 * 512:(i5 + 1) * 512], lhs,
                    qT_sb[hp][ih * 2 + i5][base:base + 64, :],
                    start=True, stop=True)
                if LDW_REUSE and not first:
                    mm.ins.ldweights = False
                first = False
            pss.append(ps)
        outs = []
        for ih, ps in enumerate(pss):
            a = attp.tile([P, 1024], bf16, name=f"att{h}_{j}_{ih}{R}",
                          tag=f"att{ih}", bufs=(14 if ih == 0 else 33))
            rsh = stat.tile([P, 1], f32, name=f"rsh{h}_{j}_{ih}{R}",
                            tag="rsh", bufs=16)
            if j % 2 == ROWSUM_DVE_PARITY:
                nc.scalar.activation(a[:], ps[:], AF.Exp, scale=SCALE)
                nc.vector.tensor_scalar(a[:], a[:], 1.0, 0.0, MULT, ADD,
                                        accum_out=rsh[:])
            else:
                nc.scalar.activation(a[:], ps[:], AF.Exp, scale=SCALE,
                                     accum_out=rsh[:])
            outs.append((a, rsh))
        return outs

    def head_finish_a(hp, hh, j, xph, outs):
        """softmax divisor onto v, then attn@V for i-half 0; saves the
        i-half-1 eT and the vsc tile for phase B."""
        h = hp * 2 + hh
        jg, jr = divmod(j, 4)
        rs = stat.tile([P, 1], f32, name=f"rs{h}_{j}{R}", tag="rs")
        nc.vector.tensor_add(rs[:], outs[0][1][:], outs[1][1][:])
        rc = stat.tile([P, 1], f32, name=f"rc{h}_{j}{R}", tag="rc")
        nc.vector.reciprocal(rc[:], rs[:])
        vsc = attp.tile([P, 64], bf16, name=f"vsc{h}_{j}{R}", tag="vsc",
                        bufs=34)
        nc.vector.tensor_scalar_mul(
            vsc[:],
            v4_sb[jg][:, jr * DL + h * 64:jr * DL + (h + 1) * 64],
            rc[:])
        vsc_t[h][j] = vsc
        a1_t[h][j] = outs[1][0]
        for i5 in range(2):
            nc.tensor.matmul(
                xph[hh * 64:(hh + 1) * 64, i5 * 512:(i5 + 1) * 512],
                vsc[:], outs[0][0][:, i5 * 512:(i5 + 1) * 512],
                start=(j == 0), stop=(j == SJ - 1),
                skip_group_check=True)

    def head_finish_b(hp, hh, j, xph):
        """attn@V for i-half 1 from the saved eT/vsc (pure PE work)."""
        h = hp * 2 + hh
        a = a1_t[h][j]
        for i5 in range(2):
            nc.tensor.matmul(
                xph[hh * 64:(hh + 1) * 64, i5 * 512:(i5 + 1) * 512],
                vsc_t[h][j][:], a[:, i5 * 512:(i5 + 1) * 512],
                start=(j == 0), stop=(j == SJ - 1),
                skip_group_check=True)

    vsc_t = [[None] * SJ for _ in range(HL)]
    a1_t = [[None] * SJ for _ in range(HL)]

    # ---------------- output projection constants ----------------
    # bo is added on the host during unshard (a K=1 ones-row matmul for it
    # here would cost 16384 PE rows ~ 7us).
    if not wo_cell:
        wo_sb = const.tile([P, DPT * DM], bf16, name="wo", tag="wo")
        nc.sync.dma_start(
            wo_sb[:].rearrange("p (t c) -> p t c", t=DPT),
            wo_d.ap().rearrange("(t p) c -> p t c", p=P))
        wo_cell.append(wo_sb)
    wo_sb = wo_cell[0]

    def emit_outproj(jts):
        for jt in jts:
            ot = outp.tile([P, DM], f32, name=f"ot{jt}{R}", tag="ot")
            ps = ppx.tile([P, DM], f32, name=f"pso{jt}{R}", tag="xps",
                          bufs=1)
            jh, jo = divmod(jt, 8)
            for cpt in range(DPT):
                first = True
                for n5 in range(2):
                    no = n5 * 512
                    mm = nc.tensor.matmul(
                        ps[:, no:no + 512],
                        xT_sb[cpt][jh][:, jo * P:(jo + 1) * P],
                        wo_sb[:, cpt * DM + no:cpt * DM + no + 512],
                        start=(cpt == 0), stop=(cpt == DPT - 1))
                    if LDW_REUSE and not first:
                        mm.ins.ldweights = False
                    first = False
            nc.vector.tensor_copy(ot[:], ps[:])
            oq = nc.sync if jt % 2 == 0 else nc.gpsimd
            oq.dma_start(O_d.ap()[jt * P:(jt + 1) * P, :], ot[:])

    def alloc_xps(hp, ih):
        return ppx.tile([P, 1024], f32, name=f"xp{ih}_{hp}{R}", tag="xps",
                        bufs=1)

    def copy_xps(hp, ih, xph):
        nc.vector.tensor_copy(xT_sb[hp][ih][:], xph[:])

    # Priority bands (lower = scheduled earlier among READY instructions).
    # The Tile list-scheduler fills each engine's idle ticks with the
    # lowest-priority ready work, so: the exp-feeding lane (scores/exp/
    # rowsum/attn@V-A) owns the rep's band; phase B (pure-PE attn@V from
    # saved tiles) fills its gaps; the PREVIOUS rep's phase-B/outproj and
    # the NEXT rep's projections (emitted in band base+0 by the caller)
    # fill what's left. Dependencies rate-limit everything, so a filler
    # can never stall the lane beyond one instruction.
    base = it * PRIO_W
    tc.cur_priority = base + 10000
    hp = 0
    xph = alloc_xps(hp, 0)
    for j in range(SJ):
        oA = head_scores(hp, 0, j)
        oB = head_scores(hp, 1, j)
        head_finish_a(hp, 0, j, xph, oA)
        head_finish_a(hp, 1, j, xph, oB)
    copy_xps(hp, 0, xph)
    # B0 feeds pair-1-A's att1 slot rotation: band just after the A1 lane
    # so it drains early in pair-1's window.
    tc.cur_priority = base + 13500
    xph = alloc_xps(hp, 1)
    for j in range(SJ):
        head_finish_b(hp, 0, j, xph)
        head_finish_b(hp, 1, j, xph)
    copy_xps(hp, 1, xph)

    hp = 1
    tc.cur_priority = base + 13000
    xph = alloc_xps(hp, 0)
    for j in range(SJ):
        oA = head_scores(hp, 0, j)
        oB = head_scores(hp, 1, j)
        head_finish_a(hp, 0, j, xph, oA)
        head_finish_a(hp, 1, j, xph, oB)
    copy_xps(hp, 0, xph)
    # B1 feeds the NEXT rep's pair-0 att1 rotation: schedule it right
    # under that rep's lane.
    tc.cur_priority = base + PRIO_W + 12000
    xph = alloc_xps(hp, 1)
    for j in range(SJ):
        head_finish_b(hp, 0, j, xph)
        head_finish_b(hp, 1, j, xph)
    copy_xps(hp, 1, xph)

    # outproj: after B1 in the next rep's window (its xT reads gate that
    # rep's first copy_xps).
    tc.cur_priority = base + PRIO_W + 12500
    # output projection: filler band, overlaps the next rep's attention
    emit_outproj(range(SJ))


def build_program(split_waits=True, reps=1):
    nc = bass.Bass("TRN2", target_bir_lowering=False, debug=False)

    qT_in = nc.dram_tensor("qT_in", [DM, S], PROJ_DT, kind="ExternalInput")
    kT_in = nc.dram_tensor("kT_in", [DM, S], PROJ_DT, kind="ExternalInput")
    vT_in = nc.dram_tensor("vT_in", [DM, S], PROJ_DT, kind="ExternalInput")
    wq_d = nc.dram_tensor("wq", [DM, DL], PROJ_DT, kind="ExternalInput")
    wk_d = nc.dram_tensor("wk", [DM, DL], PROJ_DT, kind="ExternalInput")
    wv_d = nc.dram_tensor("wv", [DM, DL], PROJ_DT, kind="ExternalInput")
    wo_d = nc.dram_tensor("wo", [DL, DM], bf16, kind="ExternalInput")
    bq_d = nc.dram_tensor("bq", [DL, 1], f32, kind="ExternalInput")
    bk_d = nc.dram_tensor("bk", [DL, 1], f32, kind="ExternalInput")
    bv_d = nc.dram_tensor("bv", [DL, 1], f32, kind="ExternalInput")
    O_d = nc.dram_tensor("O", [S, DM], f32, kind="ExternalOutput")

    with tile.TileContext(nc) as tc:
        with (
            tc.tile_pool(name="const", bufs=1) as const,
            tc.tile_pool(name="persist", bufs=1) as sb,
            tc.tile_pool(name="stat", bufs=6) as stat,
            tc.tile_pool(name="outp", bufs=3) as outp,
            tc.tile_pool(name="inp", bufs=2) as inp,
            tc.tile_pool(name="vtp", bufs=1) as vtp,
            tc.tile_pool(name="attp", bufs=20) as attp,
            tc.tile_pool(name="pps", bufs=2, space="PSUM") as pps,
            tc.tile_pool(name="ppx", bufs=1, space="PSUM") as ppx,
        ):
            # ---------------- constants ----------------
            # One DMA per weight: DRAM [(t p), c] -> SBUF [p, (t c)] so the
            # m-th 128-row block lands at free offset m*DL.
            def load_w(dram, nm, dt_, cols, eng):
                t = const.tile([P, MT * cols], dt_, name=nm, tag=nm)
                eng.dma_start(
                    t[:].rearrange("p (t c) -> p t c", t=MT),
                    dram.ap().rearrange("(t p) c -> p t c", p=P))
                return t

            wq_sb = load_w(wq_d, "wq", PROJ_DT, DL, nc.scalar)  # [128, 8*256]
            wk_sb = load_w(wk_d, "wk", PROJ_DT, DL, nc.scalar)
            bq_sb = const.tile([P, DPT], f32, name="bq", tag="bq")
            nc.sync.dma_start(
                bq_sb[:].rearrange("p (t c) -> p t c", t=DPT),
                bq_d.ap().rearrange("(t p) c -> p t c", p=P))
            bk_sb = const.tile([P, DPT], f32, name="bk", tag="bk")
            nc.sync.dma_start(
                bk_sb[:].rearrange("p (t c) -> p t c", t=DPT),
                bk_d.ap().rearrange("(t p) c -> p t c", p=P))
            bv_sb = const.tile([P, DPT], f32, name="bv", tag="bv")
            nc.sync.dma_start(
                bv_sb[:].rearrange("p (t c) -> p t c", t=DPT),
                bv_d.ap().rearrange("(t p) c -> p t c", p=P))
            wv_cell = []

            def load_wv():
                if not wv_cell:
                    wv_cell.append(load_w(wv_d, "wv", PROJ_DT, DL, nc.sync))
                return wv_cell[0]

            def w_slice(w, m, dp):
                return w[:, m * DL + dp * P:m * DL + (dp + 1) * P]

            wo_cell = []

            def mk_proj(it):
                return emit_projections(
                    nc, tc, it, const, sb, inp, vtp, ppx,
                    qT_in, kT_in, vT_in,
                    wq_sb, wk_sb, load_wv, bq_sb, bk_sb, bv_sb, w_slice)

            # rep 0's projections are the prologue; rep it+1's are
            # emitted in band (it+1)*PRIO_W + 0 so the scheduler hoists
            # them into rep it's attention as soon as their DMAs/slots
            # allow (the qT/kT/v4 double-buffering makes that legal).
            st, cl = mk_proj(0)
            tc.cur_priority = 0
            for c in cl:
                c()
            for it in range(reps):
                if it + 1 < reps:
                    st_next, cl_next = mk_proj(it + 1)
                    tc.cur_priority = it * PRIO_W + 14000
                    for c in cl_next:
                        c()
                else:
                    st_next = None
                emit_attention(nc, tc, it, st, const, stat, outp,
                               attp, pps, ppx, wo_d, O_d, wo_cell)
                st = st_next

    if LDW_REUSE:
        _dedupe_ldweights(nc)
    if split_waits:
        _split_excess_waits(nc)
    return nc


def _get_program():
    global _PROGRAM
    if _PROGRAM is None:
        _PROGRAM = build_program()
    return _PROGRAM


def _tf32(x):
    """Round fp32 -> TF32 (10-bit mantissa), round-to-nearest-even."""
    x = np.ascontiguousarray(np.asarray(x, dtype=np.float32))
    u = x.view(np.uint32)
    r = ((u >> 13) & 1).astype(np.uint32)
    u2 = ((u + np.uint32(0x0FFF) + r) & np.uint32(0xFFFFE000))
    return u2.view(np.float32)


def shard_inputs(inputs):
    """FULL inputs -> per-core in_maps (list of 8 dicts)."""
    q = np.asarray(inputs["query"], dtype=np.float32)
    k = np.asarray(inputs["key"], dtype=np.float32)
    v = np.asarray(inputs["value"], dtype=np.float32)
    Wq = np.asarray(inputs["Wq"], dtype=np.float32)
    Wk = np.asarray(inputs["Wk"], dtype=np.float32)
    Wv = np.asarray(inputs["Wv"], dtype=np.float32)
    Wo = np.asarray(inputs["Wo"], dtype=np.float32)
    bq = np.asarray(inputs["bq"], dtype=np.float32)
    bk = np.asarray(inputs["bk"], dtype=np.float32)
    bv = np.asarray(inputs["bv"], dtype=np.float32)
    bo = np.asarray(inputs["bo"], dtype=np.float32)

    if PROJ_BF16:
        import ml_dtypes

        def _proj_cast(x):
            return np.ascontiguousarray(np.asarray(x, np.float32)).astype(
                ml_dtypes.bfloat16)
    else:
        _proj_cast = _tf32

    qT = [_proj_cast(q[b].T) for b in range(B)]
    kT = [_proj_cast(k[b].T) for b in range(B)]
    vT = [_proj_cast(v[b].T) for b in range(B)]

    in_maps = []
    for c in range(N_CORES):
        b, g = c // GROUPS, c % GROUPS
        sl = slice(g * DL, (g + 1) * DL)
        in_maps.append({
            "qT_in": qT[b],
            "kT_in": kT[b],
            "vT_in": vT[b],
            "wq": _proj_cast(Wq[:, sl]),
            "wk": _proj_cast(Wk[:, sl]),
            "wv": _proj_cast(Wv[:, sl]),
            "wo": _proj_cast(Wo[sl, :]),
            "bq": np.ascontiguousarray(bq[sl].reshape(DL, 1)),
            "bk": np.ascontiguousarray(bk[sl].reshape(DL, 1)),
            "bv": np.ascontiguousarray(bv[sl].reshape(DL, 1)),
        })
    return in_maps


def unshard_output(results, bo):
    """results: list of 8 dicts with 'O' [S, DM] -> full [B, S, DM].
    bo is added here (host) -- cheaper than a K=1 PE matmul on device."""
    out = np.zeros((B, S, DM), np.float32)
    for c in range(N_CORES):
        out[c // GROUPS] += results[c]["O"]
    out += np.asarray(bo, np.float32)
    return out


def kernel(**inputs):
    nc = _get_program()
    in_maps = shard_inputs(inputs)
    res = run_bass_kernel_spmd(nc, in_maps, core_ids=list(range(N_CORES)))
    return unshard_output(res.results, inputs["bo"])



# revision 23
# speedup vs baseline: 1.5526x; 1.1659x over previous
"""Multi-head attention (softmax over query axis) on 8 Trainium2 cores.

Problem: nn_MultiHeadAttention_3899830305178
  B=2, S=2048, D_MODEL=1024, HEADS=16, D_K=64, fp32 IO.
  reference:
    q = (query @ Wq + bq), k = ..., v = ...        [b, s, h, dk]
    scores = einsum('bihd,bjhd->bijh', q, k) / 8
    attn = softmax(scores, axis=1)                 # over QUERY axis i (quirk)
    x = einsum('bijh,bjhd->bihd', attn, v)         [b, s, h*dk]
    out = x @ Wo + bo

Sharding: data-parallel over batch (2) x tensor-parallel over heads (4 groups
of 4 heads) = 8 cores. Each core computes a partial output
O_part = x_local @ Wo[rows of its heads]; the host sums the 4 partials per
batch (row-parallel unshard) -- bo is added on-device by the g==0 core.

Per-core kernel math (host passes query/key/value pre-transposed so the
projections contract over the model dim on partitions):
  qT[d', i] = Wq_s.T @ queryT      (d' = 4 local heads x 64 = 256)
  kT[d', j] = Wk_s.T @ keyT
  vT[d', j] = Wv_s.T @ valueT (+bv), then bf16 DMA-transpose -> v[j, d']
  per head h:  sT[j, i] = kT_h.T @ qT_h / 8  (softmax over i == free axis)
               eT = exp(sT) (bf16), rowsum via a DVE tensor_scalar accum
               v_h_scaled[j, :] = v_h[j, :] / rowsum[j]   <- softmax divisor
               xT_h[d, i] = v_h_scaled.T @ eT             (contracts over j)
  O_part[i, n] = xT.T @ Wo_s; bo is added on the host during unshard.

Engine balance (per CoreSim, 230us single-shot span): PE ~169us
(projections 41 + scores 55 + attn@V 55 + out-proj 14), ACT ~138us (the
128 [128,1024] exps are irreducible -- Exp exists only on ACT), DVE
~93us, 16KB/partition PSUM exactly full (2x scores buffers + 2x
proj/attn@V buffers). Startup: wq/wk load on the (initially idle) ACT
HWDGE queue in parallel with the q/k input chunks on SP; the q/k
projection stream is ordered q0,q1,k0,q2,q3,k1..k3 to match the ACT
engine's exp demand order; wv/wo load late; qT is tiled per input chunk
so each scores matmul gates on exactly one projected chunk.
Design choices vs the naive version:
  - sibling heads (partitions 0-63 / 64-127 of the kT/qT slices) emit
    their K=64 scores matmuls interleaved with PE tile positions
    (0,0)/(64,0), letting the PE overlap row tiles (K=64 alone half-fills
    the 128x128 array);
  - eT is bf16 (same PE rate as f32r, half the SBUF/attp footprint);
  - softmax rowsums come from a DVE tensor_scalar (2-byte fast mode,
    ~0.4us/tile) instead of the ACT accum_out (+187ns/tile on the other
    near-critical engine), computed as out=(a*1)+0 in place with
    accum_out=rowsum;
  - attn@V is split by i-half into [64,1024] psums so the first half's
    output projection overlaps the second half's attention (xT is split
    per i-half so the dependency is tile-precise); pair 1's first scores
    are emitted inside pair 0's phase B so the ACT engine never idles
    there, and the first-half output projection (with O DMAs alternating
    between the SP and ACT hardware queues) drains during pair 1's
    phase B;
  - the bias matmuls (K=1 ones-row) were removed from the PE: bo rides
    the host-side unshard sum.

Projection inputs/weights are bf16; scores and the output projection run
in float32r (TF32, fp32 accumulate) with fp32 softmax statistics; attn@V
is bf16 x bf16 -> fp32. Measured end-to-end relative error vs the fp64
reference is ~4.4e-3 on hardware.
"""

import numpy as np

import concourse.bass as bass
import concourse.mybir as mybir
import concourse.tile as tile
from concourse.bass_utils import run_bass_kernel_spmd

# problem shape (hardcoded per contract)
B, S, DM, H, DK = 2, 2048, 1024, 16, 64
N_CORES = 8
GROUPS = 4              # head groups (tensor-parallel)
HL = H // GROUPS        # 4 local heads per core
DL = HL * DK            # 256 local concat width
P = 128
SJ = S // P             # 16 strips of 128 along j (keys) and i (out rows)
MT = DM // P            # 8 contraction tiles for projections
DPT = DL // P           # 2 partition tiles of the local concat dim
SCALE = 1.0 / 8.0       # 1/sqrt(DK)

f32 = mybir.dt.float32
f32r = mybir.dt.float32r
bf16 = mybir.dt.bfloat16
AF = mybir.ActivationFunctionType

# Projection stage (inputs + projection weights) in bf16: halves the input
# DMA (the critical-path prefix) at ~2e-3 relative error. Attention and
# output projection stay TF32.
PROJ_BF16 = True
PROJ_DT = bf16 if PROJ_BF16 else f32r

import os as _os
# Reuse PE stationary weights across same-lhsT matmul runs by suppressing
# the per-matmul LDWEIGHTS (InstMatmult.ldweights=False on the trailing
# matmuls of each run).
LDW_REUSE = _os.environ.get("LDW_REUSE", "1") == "1"
# Which j parity routes its softmax rowsums to the DVE (the other parity
# uses the exp's ACT-side accumulator). 2 = all rowsums on ACT.
ROWSUM_DVE_PARITY = int(_os.environ.get("ROWSUM_DVE_PARITY", "1"))

PRIO_W = 1000000  # priority window per rep

_PROGRAM = None


def _dedupe_ldweights(nc):
    """Drop InstLdweights that reload the exact weights already resident.

    Tile's legalizer splits every matmul into (InstLdweights, InstMatmult
    ldweights=False); each reload costs ~120-180ns serialized into the PE
    stream. When consecutive LDWEIGHTS on the PE stream have identical
    weight APs (the kernel emits same-lhsT matmul runs for scores, attn@V
    and the output projection), the duplicates are pure overhead: the
    array still holds the weights (nothing else writes it), and the WAR
    protection on the SBUF region anchors on the matmuls (Tile tracked
    them as the lhsT readers), so dropping the reload is safe. Waits and
    sem updates of a dropped LDWEIGHTS move onto a NOP in its place."""
    n = 0
    for f in nc.m.functions:
        for blk in f.blocks:
            last_key = None
            new_insts = []
            for inst in blk.instructions:
                if getattr(inst, "engine", None) == mybir.EngineType.PE:
                    tn = type(inst).__name__
                    if tn == "InstLdweights":
                        key = (str(inst.ins[0]), str(inst.tile_position),
                               str(inst.perf_mode), str(inst.is_transpose))
                        if key == last_key:
                            si = inst.sync_info
                            if si is not None and (si.on_wait or si.on_update):
                                new_insts.append(mybir.InstNoOp(
                                    name=f"{inst.name}-ldwskip",
                                    engine=inst.engine,
                                    sync_info=si,
                                    bass_nofuse=True))
                            n += 1
                            continue
                        last_key = key
                    elif tn == "InstMatmult":
                        if inst.is_transpose:
                            last_key = None
                    elif tn in ("InstNoOp", "InstEventSemaphore"):
                        pass
                    else:
                        last_key = None
                new_insts.append(inst)
            blk.instructions[:] = new_insts
    return n


def _split_excess_waits(nc, max_waits=1):
    """walrus in this container rejects >1 semaphore wait per instruction
    (e.g. the Tile kernel-tail Drain); move extras onto same-engine NOPs."""
    n_split = 0
    for f in nc.m.functions:
        for blk in f.blocks:
            new_insts = []
            for inst in blk.instructions:
                si = getattr(inst, "sync_info", None)
                if si is not None and si.on_wait and len(si.on_wait) > max_waits:
                    waits = list(si.on_wait)
                    extra, keep = waits[:-max_waits], waits[-max_waits:]
                    for i in range(0, len(extra), max_waits):
                        chunk = extra[i:i + max_waits]
                        nop = mybir.InstNoOp(
                            name=f"{inst.name}-ws{n_split}-{i}",
                            engine=inst.engine,
                            sync_info=mybir.SyncInfo(on_wait=chunk, on_update=[]),
                            bass_nofuse=True,
                        )
                        new_insts.append(nop)
                    si.on_wait = keep
                    n_split += 1
                new_insts.append(inst)
            blk.instructions[:] = new_insts
    return n_split


QK_ORDER = [("q", 0), ("k", 0), ("q", 1), ("q", 2), ("q", 3),
            ("k", 1), ("k", 2), ("k", 3)]


def emit_projections(nc, tc, it, const, sb, inp, vtp, ppx,
                     qT_in, kT_in, vT_in,
                     wq_sb, wk_sb, load_wv, bq_sb, bk_sb, bv_sb, w_slice):
    """Create rep-it's activation tiles and return (state, chunk closures).

    Each closure emits one input chunk's DMA + projection matmuls + bias
    adds (and for v, the v4 transposes). The caller interleaves the
    closures of rep it+1 into rep it's attention j-loop so the static
    scheduler places the projection matmuls inside the attention span
    (the PE has ~2x headroom there); without this the scheduler abuts the
    reps and the ACT stream stalls ~50-60us per rep waiting for
    projections."""
    R = f"_r{it}"
    # bufs=2 on qT/kT/v4: the next rep's projections write the other
    # buffer while this rep's attention still reads this one.
    qT_sb = [[sb.tile([P, 512], bf16, name=f"qT{dp}_{i4}{R}",
                      tag=f"qT{dp}_{i4}", bufs=2) for i4 in range(4)]
             for dp in range(DPT)]
    kT_sb = [[sb.tile([P, 512], bf16, name=f"kT{dp}_{jg}{R}",
                      tag=f"kT{dp}_{jg}", bufs=2) for jg in range(4)]
             for dp in range(DPT)]
    # v packed per j-group of 4: v4_sb[jg][p, jj*DL + d'] holds
    # v[jg*512 + jj*128 + p, d']
    v4_sb = [sb.tile([P, 4 * DL], bf16, name=f"v{jg}{R}", tag=f"v{jg}",
                     bufs=2)
             for jg in range(4)]
    xT_sb = [[sb.tile([P, 1024], bf16, name=f"xT{hp}_{ih}{R}",
                      tag=f"xT{hp}_{ih}") for ih in range(2)]
             for hp in range(DPT)]
    vT_sb = [vtp.tile([P, S], bf16, name=f"vT{dp}{R}", tag=f"vT{dp}")
             for dp in range(DPT)]

    def load_in_chunk(win, nm, i4):
        # one DMA: all 8 m-blocks of columns [i0, i0+512)
        t = inp.tile([P, MT * 512], PROJ_DT, name=f"{nm}in{i4}{R}",
                     tag="pin")
        src = win.ap().rearrange("(t p) c -> p t c", p=P)
        nc.sync.dma_start(
            t[:].rearrange("p (t c) -> p t c", t=MT),
            src[:, :, i4 * 512:(i4 + 1) * 512])
        return t

    def qk_chunk(nm, i4):
        def emit():
            win, w_sb, b_sb = ((qT_in, wq_sb, bq_sb) if nm == "q"
                               else (kT_in, wk_sb, bk_sb))
            ch = load_in_chunk(win, nm, i4)
            for dp in range(DPT):
                ps = ppx.tile([P, 512], f32, name=f"ps{nm}{i4}_{dp}{R}",
                              tag="px", bufs=2)
                for m in range(MT):
                    nc.tensor.matmul(
                        ps[:], w_slice(w_sb, m, dp),
                        ch[:, m * 512:(m + 1) * 512],
                        start=(m == 0), stop=(m == MT - 1))
                dst = (qT_sb if nm == "q" else kT_sb)[dp][i4][:]
                nc.vector.tensor_scalar_add(dst, ps[:], b_sb[:, dp:dp + 1])
        return emit

    def v_chunk(i4):
        def emit():
            wv_sb = load_wv()
            i0 = i4 * 512
            ch = load_in_chunk(vT_in, "v", i4)
            for dp in range(DPT):
                ps = ppx.tile([P, 512], f32, name=f"psvt{i4}_{dp}{R}",
                              tag="px", bufs=2)
                for m in range(MT):
                    nc.tensor.matmul(
                        ps[:], w_slice(wv_sb, m, dp),
                        ch[:, m * 512:(m + 1) * 512],
                        start=(m == 0), stop=(m == MT - 1))
                nc.vector.tensor_scalar_add(
                    vT_sb[dp][:, i0:i0 + 512], ps[:], bv_sb[:, dp:dp + 1])
            for dp in range(DPT):
                out_view = v4_sb[i4][:].rearrange(
                    "p (j c) -> p j c", j=4)[:, :, dp * P:(dp + 1) * P]
                nc.sync.dma_start(
                    out_view, vT_sb[dp][:, i0:i0 + 512], transpose=True)
        return emit

    closures = [qk_chunk(nm, i4) for nm, i4 in QK_ORDER]
    closures += [v_chunk(i4) for i4 in range(4)]
    st = dict(qT_sb=qT_sb, kT_sb=kT_sb, v4_sb=v4_sb, xT_sb=xT_sb)
    return st, closures


def emit_attention(nc, tc, it, st, const, stat, outp, attp, pps,
                   ppx, wo_d, O_d, wo_cell):
    """Attention + output projection for rep it; `feeder` holds the next
    rep's projection-chunk closures, interleaved into the j-loop."""
    R = f"_r{it}"
    qT_sb, kT_sb = st["qT_sb"], st["kT_sb"]
    v4_sb, xT_sb = st["v4_sb"], st["xT_sb"]

    # ---------------- attention ----------------
    # Heads run in sibling pairs (2hp, 2hp+1) whose kT/qT slices live at
    # partitions 0-63 / 64-127 (PE row tiles 0/64). Per (head, j) the four
    # scores matmuls (2 i-halves x 2 i-chunks) share one kT stationary
    # slice and the four attn@V matmuls share one vsc slice: with
    # LDW_REUSE the trailing matmuls set InstMatmult.ldweights=False so
    # walrus skips the per-matmul LDWEIGHTS reload (HW trace showed the
    # 768 reloads/rep serialize ~130ns each into the PE stream).
    # Softmax rowsums ride the exp's ACT-side accumulator on even j and a
    # DVE tensor_scalar on odd j, balancing the two near-critical engines.
    # attn@V accumulates both i-halves into two [128, 1024] psums held for
    # the whole pair (sibling heads at psum partitions 0-63/64-127).

    MULT = mybir.AluOpType.mult
    ADD = mybir.AluOpType.add

    def head_scores(hp, hh, j):
        """scores + exp + rowsum for one head, full i range (2 psum tiles)."""
        jg, jr = divmod(j, 4)
        h = hp * 2 + hh
        base = hh * 64
        lhs = kT_sb[hp][jg][base:base + 64, jr * P:(jr + 1) * P]
        pss = []
        first = True
        for ih in range(2):
            ps = pps.tile([P, 1024], f32, name=f"ps{h}_{j}_{ih}{R}",
                          tag="ps")
            for i5 in range(2):
                mm = nc.tensor.matmul(
                    ps[:, i5 * 512:(i5 + 1) * 512], lhs,
                    qT_sb[hp][ih * 2 + i5][base:base + 64, :],
                    start=True, stop=True)
                if LDW_REUSE and not first:
                    mm.ins.ldweights = False
                first = False
            pss.append(ps)
        outs = []
        for ih, ps in enumerate(pss):
            a = attp.tile([P, 1024], bf16, name=f"att{h}_{j}_{ih}{R}",
                          tag=f"att{ih}", bufs=(14 if ih == 0 else 33))
            rsh = stat.tile([P, 1], f32, name=f"rsh{h}_{j}_{ih}{R}",
                            tag="rsh", bufs=16)
            if j % 2 == ROWSUM_DVE_PARITY:
                nc.scalar.activation(a[:], ps[:], AF.Exp, scale=SCALE)
                nc.vector.tensor_scalar(a[:], a[:], 1.0, 0.0, MULT, ADD,
                                        accum_out=rsh[:])
            else:
                nc.scalar.activation(a[:], ps[:], AF.Exp, scale=SCALE,
                                     accum_out=rsh[:])
            outs.append((a, rsh))
        return outs

    def head_finish_a(hp, hh, j, xph, outs):
        """softmax divisor onto v, then attn@V for i-half 0; saves the
        i-half-1 eT and the vsc tile for phase B."""
        h = hp * 2 + hh
        jg, jr = divmod(j, 4)
        rs = stat.tile([P, 1], f32, name=f"rs{h}_{j}{R}", tag="rs")
        nc.vector.tensor_add(rs[:], outs[0][1][:], outs[1][1][:])
        rc = stat.tile([P, 1], f32, name=f"rc{h}_{j}{R}", tag="rc")
        nc.vector.reciprocal(rc[:], rs[:])
        vsc = attp.tile([P, 64], bf16, name=f"vsc{h}_{j}{R}", tag="vsc",
                        bufs=34)
        nc.vector.tensor_scalar_mul(
            vsc[:],
            v4_sb[jg][:, jr * DL + h * 64:jr * DL + (h + 1) * 64],
            rc[:])
        vsc_t[h][j] = vsc
        a1_t[h][j] = outs[1][0]
        for i5 in range(2):
            nc.tensor.matmul(
                xph[hh * 64:(hh + 1) * 64, i5 * 512:(i5 + 1) * 512],
                vsc[:], outs[0][0][:, i5 * 512:(i5 + 1) * 512],
                start=(j == 0), stop=(j == SJ - 1),
                skip_group_check=True)

    def head_finish_b(hp, hh, j, xph):
        """attn@V for i-half 1 from the saved eT/vsc (pure PE work)."""
        h = hp * 2 + hh
        a = a1_t[h][j]
        for i5 in range(2):
            nc.tensor.matmul(
                xph[hh * 64:(hh + 1) * 64, i5 * 512:(i5 + 1) * 512],
                vsc_t[h][j][:], a[:, i5 * 512:(i5 + 1) * 512],
                start=(j == 0), stop=(j == SJ - 1),
                skip_group_check=True)

    vsc_t = [[None] * SJ for _ in range(HL)]
    a1_t = [[None] * SJ for _ in range(HL)]

    # ---------------- output projection constants ----------------
    # bo is added on the host during unshard (a K=1 ones-row matmul for it
    # here would cost 16384 PE rows ~ 7us).
    if not wo_cell:
        wo_sb = const.tile([P, DPT * DM], bf16, name="wo", tag="wo")
        nc.sync.dma_start(
            wo_sb[:].rearrange("p (t c) -> p t c", t=DPT),
            wo_d.ap().rearrange("(t p) c -> p t c", p=P))
        wo_cell.append(wo_sb)
    wo_sb = wo_cell[0]

    def emit_outproj(jts):
        for jt in jts:
            ot = outp.tile([P, DM], f32, name=f"ot{jt}{R}", tag="ot")
            jh, jo = divmod(jt, 8)
            # two [128,512] psums (the two n-halves) live on the px slots;
            # cpt-outer order keeps same-lhsT matmul runs for LDW reuse.
            pss = [ppx.tile([P, 512], f32, name=f"pso{jt}_{n5}{R}",
                            tag="px", bufs=2) for n5 in range(2)]
            for cpt in range(DPT):
                first = True
                for n5 in range(2):
                    mm = nc.tensor.matmul(
                        pss[n5][:],
                        xT_sb[cpt][jh][:, jo * P:(jo + 1) * P],
                        wo_sb[:, cpt * DM + n5 * 512:cpt * DM + n5 * 512 + 512],
                        start=(cpt == 0), stop=(cpt == DPT - 1))
                    if LDW_REUSE and not first:
                        mm.ins.ldweights = False
                    first = False
            for n5 in range(2):
                nc.vector.tensor_copy(ot[:, n5 * 512:(n5 + 1) * 512],
                                      pss[n5][:])
            oq = nc.sync if jt % 2 == 0 else nc.gpsimd
            oq.dma_start(O_d.ap()[jt * P:(jt + 1) * P, :], ot[:])

    def alloc_xps(hp, ih):
        return ppx.tile([P, 1024], f32, name=f"xp{ih}_{hp}{R}", tag="xps",
                        bufs=1)

    def copy_xps(hp, ih, xph):
        nc.vector.tensor_copy(xT_sb[hp][ih][:], xph[:])

    # Priority bands (lower = scheduled earlier among READY instructions).
    # The Tile list-scheduler fills each engine's idle ticks with the
    # lowest-priority ready work, so: the exp-feeding lane (scores/exp/
    # rowsum/attn@V-A) owns the rep's band; phase B (pure-PE attn@V from
    # saved tiles) fills its gaps; the PREVIOUS rep's phase-B/outproj and
    # the NEXT rep's projections (emitted in band base+0 by the caller)
    # fill what's left. Dependencies rate-limit everything, so a filler
    # can never stall the lane beyond one instruction.
    base = it * PRIO_W
    tc.cur_priority = base + 10000
    hp = 0
    xph = alloc_xps(hp, 0)
    for j in range(SJ):
        oA = head_scores(hp, 0, j)
        oB = head_scores(hp, 1, j)
        head_finish_a(hp, 0, j, xph, oA)
        head_finish_a(hp, 1, j, xph, oB)
    copy_xps(hp, 0, xph)
    # B0 feeds pair-1-A's att1 slot rotation: band just after the A1 lane
    # so it drains early in pair-1's window.
    tc.cur_priority = base + 13500
    xph = alloc_xps(hp, 1)
    for j in range(SJ):
        head_finish_b(hp, 0, j, xph)
        head_finish_b(hp, 1, j, xph)
    copy_xps(hp, 1, xph)

    hp = 1
    tc.cur_priority = base + 13000
    xph = alloc_xps(hp, 0)
    for j in range(SJ):
        oA = head_scores(hp, 0, j)
        oB = head_scores(hp, 1, j)
        head_finish_a(hp, 0, j, xph, oA)
        head_finish_a(hp, 1, j, xph, oB)
    copy_xps(hp, 0, xph)
    # B1 feeds the NEXT rep's pair-0 att1 rotation: schedule it right
    # under that rep's lane.
    tc.cur_priority = base + PRIO_W + 11000
    xph = alloc_xps(hp, 1)
    for j in range(SJ):
        head_finish_b(hp, 0, j, xph)
        head_finish_b(hp, 1, j, xph)
    copy_xps(hp, 1, xph)

    # outproj: after B1 in the next rep's window (its xT reads gate that
    # rep's first copy_xps).
    tc.cur_priority = base + PRIO_W + 11500
    # output projection: filler band, overlaps the next rep's attention
    emit_outproj(range(SJ))


def build_program(split_waits=True, reps=1):
    nc = bass.Bass("TRN2", target_bir_lowering=False, debug=False)

    qT_in = nc.dram_tensor("qT_in", [DM, S], PROJ_DT, kind="ExternalInput")
    kT_in = nc.dram_tensor("kT_in", [DM, S], PROJ_DT, kind="ExternalInput")
    vT_in = nc.dram_tensor("vT_in", [DM, S], PROJ_DT, kind="ExternalInput")
    wq_d = nc.dram_tensor("wq", [DM, DL], PROJ_DT, kind="ExternalInput")
    wk_d = nc.dram_tensor("wk", [DM, DL], PROJ_DT, kind="ExternalInput")
    wv_d = nc.dram_tensor("wv", [DM, DL], PROJ_DT, kind="ExternalInput")
    wo_d = nc.dram_tensor("wo", [DL, DM], bf16, kind="ExternalInput")
    bq_d = nc.dram_tensor("bq", [DL, 1], f32, kind="ExternalInput")
    bk_d = nc.dram_tensor("bk", [DL, 1], f32, kind="ExternalInput")
    bv_d = nc.dram_tensor("bv", [DL, 1], f32, kind="ExternalInput")
    O_d = nc.dram_tensor("O", [S, DM], f32, kind="ExternalOutput")

    with tile.TileContext(nc) as tc:
        with (
            tc.tile_pool(name="const", bufs=1) as const,
            tc.tile_pool(name="persist", bufs=1) as sb,
            tc.tile_pool(name="stat", bufs=6) as stat,
            tc.tile_pool(name="outp", bufs=3) as outp,
            tc.tile_pool(name="inp", bufs=2) as inp,
            tc.tile_pool(name="vtp", bufs=1) as vtp,
            tc.tile_pool(name="attp", bufs=20) as attp,
            tc.tile_pool(name="pps", bufs=2, space="PSUM") as pps,
            tc.tile_pool(name="ppx", bufs=1, space="PSUM") as ppx,
        ):
            # ---------------- constants ----------------
            # One DMA per weight: DRAM [(t p), c] -> SBUF [p, (t c)] so the
            # m-th 128-row block lands at free offset m*DL.
            def load_w(dram, nm, dt_, cols, eng):
                t = const.tile([P, MT * cols], dt_, name=nm, tag=nm)
                eng.dma_start(
                    t[:].rearrange("p (t c) -> p t c", t=MT),
                    dram.ap().rearrange("(t p) c -> p t c", p=P))
                return t

            wq_sb = load_w(wq_d, "wq", PROJ_DT, DL, nc.scalar)  # [128, 8*256]
            wk_sb = load_w(wk_d, "wk", PROJ_DT, DL, nc.scalar)
            bq_sb = const.tile([P, DPT], f32, name="bq", tag="bq")
            nc.sync.dma_start(
                bq_sb[:].rearrange("p (t c) -> p t c", t=DPT),
                bq_d.ap().rearrange("(t p) c -> p t c", p=P))
            bk_sb = const.tile([P, DPT], f32, name="bk", tag="bk")
            nc.sync.dma_start(
                bk_sb[:].rearrange("p (t c) -> p t c", t=DPT),
                bk_d.ap().rearrange("(t p) c -> p t c", p=P))
            bv_sb = const.tile([P, DPT], f32, name="bv", tag="bv")
            nc.sync.dma_start(
                bv_sb[:].rearrange("p (t c) -> p t c", t=DPT),
                bv_d.ap().rearrange("(t p) c -> p t c", p=P))
            wv_cell = []

            def load_wv():
                if not wv_cell:
                    wv_cell.append(load_w(wv_d, "wv", PROJ_DT, DL, nc.sync))
                return wv_cell[0]

            def w_slice(w, m, dp):
                return w[:, m * DL + dp * P:m * DL + (dp + 1) * P]

            wo_cell = []

            def mk_proj(it):
                return emit_projections(
                    nc, tc, it, const, sb, inp, vtp, ppx,
                    qT_in, kT_in, vT_in,
                    wq_sb, wk_sb, load_wv, bq_sb, bk_sb, bv_sb, w_slice)

            # rep 0's projections are the prologue; rep it+1's are
            # emitted in band (it+1)*PRIO_W + 0 so the scheduler hoists
            # them into rep it's attention as soon as their DMAs/slots
            # allow (the qT/kT/v4 double-buffering makes that legal).
            st, cl = mk_proj(0)
            tc.cur_priority = 0
            for c in cl:
                c()
            for it in range(reps):
                if it + 1 < reps:
                    st_next, cl_next = mk_proj(it + 1)
                    tc.cur_priority = it * PRIO_W + 14000
                    for c in cl_next:
                        c()
                else:
                    st_next = None
                emit_attention(nc, tc, it, st, const, stat, outp,
                               attp, pps, ppx, wo_d, O_d, wo_cell)
                st = st_next

    if LDW_REUSE:
        _dedupe_ldweights(nc)
    if split_waits:
        _split_excess_waits(nc)
    return nc


def _get_program():
    global _PROGRAM
    if _PROGRAM is None:
        _PROGRAM = build_program()
    return _PROGRAM


def _tf32(x):
    """Round fp32 -> TF32 (10-bit mantissa), round-to-nearest-even."""
    x = np.ascontiguousarray(np.asarray(x, dtype=np.float32))
    u = x.view(np.uint32)
    r = ((u >> 13) & 1).astype(np.uint32)
    u2 = ((u + np.uint32(0x0FFF) + r) & np.uint32(0xFFFFE000))
    return u2.view(np.float32)


def shard_inputs(inputs):
    """FULL inputs -> per-core in_maps (list of 8 dicts)."""
    q = np.asarray(inputs["query"], dtype=np.float32)
    k = np.asarray(inputs["key"], dtype=np.float32)
    v = np.asarray(inputs["value"], dtype=np.float32)
    Wq = np.asarray(inputs["Wq"], dtype=np.float32)
    Wk = np.asarray(inputs["Wk"], dtype=np.float32)
    Wv = np.asarray(inputs["Wv"], dtype=np.float32)
    Wo = np.asarray(inputs["Wo"], dtype=np.float32)
    bq = np.asarray(inputs["bq"], dtype=np.float32)
    bk = np.asarray(inputs["bk"], dtype=np.float32)
    bv = np.asarray(inputs["bv"], dtype=np.float32)
    bo = np.asarray(inputs["bo"], dtype=np.float32)

    if PROJ_BF16:
        import ml_dtypes

        def _proj_cast(x):
            return np.ascontiguousarray(np.asarray(x, np.float32)).astype(
                ml_dtypes.bfloat16)
    else:
        _proj_cast = _tf32

    qT = [_proj_cast(q[b].T) for b in range(B)]
    kT = [_proj_cast(k[b].T) for b in range(B)]
    vT = [_proj_cast(v[b].T) for b in range(B)]

    in_maps = []
    for c in range(N_CORES):
        b, g = c // GROUPS, c % GROUPS
        sl = slice(g * DL, (g + 1) * DL)
        in_maps.append({
            "qT_in": qT[b],
            "kT_in": kT[b],
            "vT_in": vT[b],
            "wq": _proj_cast(Wq[:, sl]),
            "wk": _proj_cast(Wk[:, sl]),
            "wv": _proj_cast(Wv[:, sl]),
            "wo": _proj_cast(Wo[sl, :]),
            "bq": np.ascontiguousarray(bq[sl].reshape(DL, 1)),
            "bk": np.ascontiguousarray(bk[sl].reshape(DL, 1)),
            "bv": np.ascontiguousarray(bv[sl].reshape(DL, 1)),
        })
    return in_maps


def unshard_output(results, bo):
    """results: list of 8 dicts with 'O' [S, DM] -> full [B, S, DM].
    bo is added here (host) -- cheaper than a K=1 PE matmul on device."""
    out = np.zeros((B, S, DM), np.float32)
    for c in range(N_CORES):
        out[c // GROUPS] += results[c]["O"]
    out += np.asarray(bo, np.float32)
    return out


def kernel(**inputs):
    nc = _get_program()
    in_maps = shard_inputs(inputs)
    res = run_bass_kernel_spmd(nc, in_maps, core_ids=list(range(N_CORES)))
    return unshard_output(res.results, inputs["bo"])

